# revision 61
# baseline (speedup 1.0000x reference)
"""Causal MHA (batch=4, seq=2048, dim=1024, 16 heads x 64) on 8 TRN2 NeuronCores.

Sharding: core c handles batch b = c//2 and head-group g = c%2 (8 heads).
Each core computes QKV projections for its heads, causal attention, and a
partial output projection over its 512 features. The host sums the two
partial projections per batch and transposes back.

All matmuls run in bf16 (fp32 PSUM accumulate); softmax runs without max
subtraction (logits are bounded ~|8|), with the row sums produced by an
extra ones-column appended to V during the PV matmul. The causal mask on
diagonal S^T blocks is a post-exp DVE multiply by a 0/1 triangle (off
the PE). The recip chain (ln, exp(-x)) runs on ACT; rank-1 matmuls
broadcast the recip rows and DVE normalizes O^T in place two blocks
later.

Scheduling notes (hard-won on HW):
- The chip power-governor downclocks ~20% chip-wide (2.4->2.0GHz) if the
  8 cores pack engines or DMA too densely; v1-level density at full
  clock beats denser schedules. Fine-grained strided DMA (1KB lines)
  also trips it — all transfers use coarse contiguous lines, with xT
  host-preshuffled into token-quarter-major layout.
- Inputs land as per-di [wq|wk|wv] tiles on the two HW rings and
  token-quarter x tiles (SWDGE + rings), so the first QKV chain is
  DMA-paced from ~11us. Q/K live in per-chunk tiles; later pairs'
  projections are emitted as per-chunk fill units, deferred as late as
  consumers allow so PE filler reaches the otherwise-starved
  (2,2)/(3,*) blocks (this halved the HAM re-throttle time).
- Per (head-pair, q-chunk), sims stream at the exp pace via the
  2-buffer sim-PSUM round-robin; PV, V tiles, projection columns and
  the deferred Q/K parts fill the PE between them. Diagonal tiles
  stream only their valid q columns on both the sim and PV matmuls.
- Outputs are cast into 2-row-block staging tiles and DMA'd in batched
  transfers on the sync/SWDGE queues, keeping the ACT queue free of
  DMA issues and the tail drain short.
"""
import sys

sys.path.insert(0, "/opt/trn_rl_repo")

import json
import numpy as np
import ml_dtypes
from contextlib import ExitStack

import concourse.bass as bass
import concourse.tile as tile
from concourse import mybir
from concourse import bass_utils as _bu
from concourse.bass_utils import run_bass_kernel_spmd

LDW_OPT = False  # walrus ldw-opt rejects bass-emitted Ldweights outright

BF16 = mybir.dt.bfloat16
F32 = mybir.dt.float32
F32R = mybir.dt.float32r
Exp = mybir.ActivationFunctionType.Exp
Ln = mybir.ActivationFunctionType.Ln

DIM = 1024
SEQ = 2048
NH = 16          # total heads
HPC = 8          # heads per core
DH = 64          # head dim
SCALE = DH ** -0.5
NCORES = 8
FPC = HPC * DH   # features per core = 512
NKT = SEQ // 128   # 16 k-tiles of 128
NQC = SEQ // 512   # 4 q-chunks of 512
VSTRIDE = DH + 2   # 66: V columns per head incl. ones col + pad

_WALRUS_PATCHED = False


def _patch_walrus_wait_limit():
    """This container's walrus rejects >1 sem wait per instruction
    (CoreV3 setupSyncWait). Tile's tail drain carries one wait per live
    proc; split the extras into preceding single-wait Drain carriers at
    BIR-JSON serialization time."""
    global _WALRUS_PATCHED
    if _WALRUS_PATCHED:
        return
    _WALRUS_PATCHED = True

    if LDW_OPT:
        orig_run = _bu.run_command

        def run_patched(cmd, *a, **k):
            cmd = ["--enable-ldw-opt=true" if c == "--enable-ldw-opt=false" else c
                   for c in cmd]
            return orig_run(cmd, *a, **k)

        _bu.run_command = run_patched

    orig = bass.Bass.to_json_bytes

    def _merge_ldw_halves(insts):
        """Fold row-tiled Ldweights pairs ([64,128] at row 0 + [64,128] at
        row 64 of the same tensor) into one [128,128] load carrying both
        halves' waits."""
        out = []
        pend = None  # (index_in_out, inst) of a candidate row-0 half
        for inst in insts:
            op = inst["opcode"]
            if inst.get("engine") != "PE":
                out.append(inst)
                continue
            if op == "Ldweights" and inst.get("tile_size") == [64, 128]:
                ap = inst["ins"][0].get("ap")
                if inst.get("tile_position") == [0, 0] and ap and ap[0][1] == 64:
                    out.append(inst)
                    pend = (len(out) - 1, inst)
                    continue
                if (pend is not None
                        and inst.get("tile_position") == [64, 0] and ap
                        and ap[0][1] == 64):
                    a = pend[1]
                    aap = a["ins"][0]["ap"]
                    same = (a["ins"][0].get("memref") == inst["ins"][0].get("memref")
                            and aap[0][0] == ap[0][0] and aap[1] == ap[1]
                            and inst["ins"][0].get("offset", 0)
                            == a["ins"][0].get("offset", 0) + 64 * aap[0][0])
                    b_si = inst.get("sync_info") or {}
                    if same and not b_si.get("on_update"):
                        aap[0][1] = 128
                        a["tile_size"] = [128, 128]
                        a.setdefault("sync_info", {"on_update": [], "on_wait": []})
                        a["sync_info"].setdefault("on_wait", [])
                        a["sync_info"]["on_wait"].extend(b_si.get("on_wait") or [])
                        pend = None
                        continue
                out.append(inst)
                pend = None
            else:
                if op not in ("Matmult", "NoOp"):
                    pend = None
                out.append(inst)
        return out

    def patched(self, *a, **k):
        d = json.loads(orig(self, *a, **k))
        for f in d["functions"]:
            for bb in f["blocks"]:
                bb["instructions"] = _merge_ldw_halves(bb["instructions"])
                out = []
                last_ldw = None  # (key, still_valid)
                for inst in bb["instructions"]:
                    si = inst.get("sync_info")
                    ow = (si or {}).get("on_wait") or []
                    op = inst["opcode"]

                    def emit_carriers(waits):
                        for j, w in enumerate(waits):
                            out.append({
                                "name": f"{inst['name']}__w{j}",
                                "opcode": "NoOp",
                                "engine": inst["engine"],
                                "ins": [], "outs": [],
                                "debug": inst.get("debug", 0),
                                "sync_info": {"on_update": [], "on_wait": [w]},
                            })

                    # drop a Ldweights identical to the previous one when only
                    # Matmult/NoOp sit between (weights already resident);
                    # also fold the row-tiled [64,128]+[64,128] half-pair into
                    # the single [128,128] load emitted by _merge_ldw_halves
                    if op == "Ldweights" and inst["engine"] == "PE":
                        key = json.dumps(
                            [inst.get("ins"), inst.get("tile_position"),
                             inst.get("tile_size")], sort_keys=True)
                        if last_ldw == key and not (si or {}).get("on_update"):
                            emit_carriers(ow)
                            continue
                        last_ldw = key
                    elif inst["engine"] == "PE" and op not in ("Matmult", "NoOp"):
                        last_ldw = None

                    if len(ow) > 1:
                        emit_carriers(ow[:-1])
                        si["on_wait"] = [ow[-1]]
                    out.append(inst)
                bb["instructions"] = out
        return json.dumps(d).encode()

    bass.Bass.to_json_bytes = patched


def build_kernel():
    nc = bass.Bass()
    # host packs xT quarter-major: [tq*128+p, di*512+t] so each token
    # quarter is one contiguous [128, 4096] transfer with 4KB lines
    xT = nc.declare_dram_parameter("xT", [4 * 128, 8 * 512], BF16,
                                   isOutput=False)
    # wq|wk per-di rows (the first chains' critical bytes); wv separately,
    # host-packed [p, di*512+c] so it is one contiguous 8KB-line transfer
    wqk = nc.declare_dram_parameter("wqk", [DIM, 2 * FPC], BF16,
                                    isOutput=False)
    wv = nc.declare_dram_parameter("wv", [128, 8 * FPC], BF16,
                                   isOutput=False)
    wo = nc.declare_dram_parameter("wo", [FPC, DIM], BF16, isOutput=False)
    # tri = inclusive lower-triangular 0/1 mask; the diagonal S^T block is
    # exp'd unmasked (logits are bounded, no overflow) and the above-diag
    # entries are zeroed by a DVE multiply, keeping the mask off the PE
    tri = nc.declare_dram_parameter("tri", [128, 128], BF16, isOutput=False)
    outT = nc.declare_dram_parameter("outT", [DIM, SEQ], BF16, isOutput=True)

    with tile.TileContext(nc) as tc, ExitStack() as ctx:
        persist = ctx.enter_context(tc.tile_pool(name="persist", bufs=1))
        work = ctx.enter_context(tc.tile_pool(name="work", bufs=4))
        pt_pool = ctx.enter_context(tc.tile_pool(name="pt", bufs=1))
        ps_mm = ctx.enter_context(tc.tile_pool(name="ps_mm", bufs=2, space="PSUM"))
        ps_s = ctx.enter_context(tc.tile_pool(name="ps_s", bufs=2, space="PSUM"))
        ps_o = ctx.enter_context(tc.tile_pool(name="ps_o", bufs=2, space="PSUM"))

        # ---- load inputs. Fine-grained tiles so consumers start as soon
        # as their own bytes land: weights as per-di [wq|wk|wv] tiles
        # (contiguous 3KB lines) split across the two HW rings, xT as 4
        # host-preshuffled token-quarter tiles (contiguous 4KB lines),
        # Q/K as per-chunk tiles. The first QKV chains + sims are then
        # DMA-paced from ~13us instead of waiting for whole tensors. All
        # transfers keep coarse contiguous lines (fine-grained strided
        # patterns measurably downclock the chip via the power governor).
        w_sb = [persist.tile([128, 2 * FPC], BF16, tag=f"w{di}",
                             name=f"w{di}") for di in range(8)]
        wv_wide = persist.tile([128, 8 * FPC], BF16, tag="wv", name="wv")
        xq_sb = [persist.tile([128, 8 * 512], BF16, tag=f"xq{tq}",
                              name=f"xq{tq}") for tq in range(4)]
        wo_wide = persist.tile([128, 4 * DIM], BF16, tag="wo", name="wo")
        tri_sb = persist.tile([128, 128], BF16, tag="tri", name="tri")
        def wsl(name, di):       # [128,FPC] view of weight block di
            if name == "wv":
                return wv_wide[:, di * FPC:(di + 1) * FPC]
            return w_sb[di][:, 0:FPC] if name == "wq" else w_sb[di][:, FPC:]

        def xsl(di, tq):         # [128,512] token-quarter tq of x block di
            return xq_sb[tq][:, di * 512:(di + 1) * 512]

        def w_dma(eng, di):
            eng.dma_start(w_sb[di][:],
                          wqk.ap()[di * 128:(di + 1) * 128, :])

        def x_dma(eng, tq):
            eng.dma_start(xq_sb[tq][:],
                          xT.ap()[tq * 128:(tq + 1) * 128, :])

        # all weight blocks land by ~24us on the two HW rings; xq0 rides
        # the SWDGE front so the first chain has data ~20us; later token
        # quarters trail (their consumers are fills in later blocks)
        nc.scalar.dma_start(tri_sb[:], tri.ap())
        nc.gpsimd.dma_start(xq_sb[0][:], xT.ap()[0 * 128:1 * 128, :])
        w_dma(nc.sync, 0)
        w_dma(nc.scalar, 1)
        w_dma(nc.sync, 2)
        w_dma(nc.scalar, 3)
        w_dma(nc.sync, 4)
        w_dma(nc.scalar, 5)
        w_dma(nc.sync, 6)
        w_dma(nc.scalar, 7)
        nc.gpsimd.dma_start(wv_wide[:], wv.ap())
        x_dma(nc.scalar, 1)
        nc.gpsimd.dma_start(xq_sb[2][:], xT.ap()[2 * 128:3 * 128, :])
        x_dma(nc.sync, 3)
        nc.gpsimd.dma_start(
            wo_wide[:].rearrange("p (fi c) -> p fi c", fi=4),
            wo.ap().rearrange("(fi p) c -> p fi c", fi=4))
        wo_sb = [wo_wide[:, fi * DIM:(fi + 1) * DIM] for fi in range(4)]
        ones64 = persist.tile([1, DH], BF16, tag="ones64")
        nc.gpsimd.memset(ones64[:], 1.0)

        # ---- stage B: QKV projections -----------------------------------
        qk_sb = {"q": [], "k": []}
        for qn in ("q", "k"):
            for fi in range(4):
                qk_sb[qn].append(
                    [persist.tile([128, 512], BF16, tag=f"{qn}{fi}t{tck}",
                                  name=f"{qn}{fi}t{tck}") for tck in range(4)])
        v_sb = [persist.tile([128, HPC * VSTRIDE], BF16, tag=f"v{ti}",
                             name=f"v{ti}") for ti in range(NKT)]

        def emit_qk_part(qn, fi, tck):
            # one token-chunk of a pair's Q or K projection (fill unit)
            wn = "wq" if qn == "q" else "wk"
            ch = ps_mm.tile([128, 512], F32, tag="mm", name="qkp")
            for di in range(8):
                nc.tensor.matmul(
                    ch[:], wsl(wn, di)[:, fi * 128:(fi + 1) * 128],
                    xsl(di, tck),
                    start=(di == 0), stop=(di == 7))
            nc.vector.tensor_copy(qk_sb[qn][fi][tck][:], ch[:])

        def emit_v(ti):
            # V in [token, feature] layout (xT stationary, wv moving), strided
            # into VSTRIDE-blocks with a ones column per head
            t = v_sb[ti]
            p = ps_mm.tile([128, 512], F32, tag="mm", name="p_v")
            for di in range(8):
                nc.tensor.matmul(
                    p[:], xsl(di, ti // 4)[:, (ti % 4) * 128:(ti % 4 + 1) * 128],
                    wsl("wv", di),
                    start=(di == 0), stop=(di == 7))
            dst = t[:].rearrange("p (h c) -> p h c", h=HPC)[:, :, 0:DH]
            src = p[:].rearrange("p (h c) -> p h c", h=HPC)
            nc.vector.tensor_copy(dst, src)
            nc.gpsimd.memset(
                t[:].rearrange("p (h c) -> p h c", h=HPC)[:, :, DH:DH + 1], 1.0)

        ot_sb = [persist.tile([128, SEQ], BF16, tag=f"ot{fi}", name=f"ot{fi}")
                 for fi in range(4)]
        pts_map = {}

        def emit_sim(pr, ci, j0, j1):
            # S^T strips + exp into pt tiles for (head pair pr, q-chunk ci),
            # k-tiles j0..j1-1. Diagonal tiles (r >= 1) stream only their
            # valid q columns.
            q0 = ci * 512
            pts = pts_map.setdefault((pr, ci), {})
            for j in range(j0, j1):
                r = j - 4 * ci
                c0 = 128 * r if r > 0 else 0   # first valid q col in chunk
                ps = ps_s.tile([128, 1024], F32, tag="s", name="ps_st")
                for half in range(2):   # head A / head B, row-tiled
                    nc.tensor.matmul(
                        ps[:, half * 512 + c0:(half + 1) * 512],
                        qk_sb["k"][pr][j // 4][half * 64:(half + 1) * 64,
                                               (j % 4) * 128:(j % 4 + 1) * 128],
                        qk_sb["q"][pr][ci][half * 64:(half + 1) * 64,
                                           c0:512],
                        start=True, stop=True)
                pt = pt_pool.tile([128, 1024], BF16, tag=f"pt{j}", name="pt",
                                  bufs=2 if j < 14 else 1)
                pts[j] = pt
                if r < 0:
                    nc.scalar.activation(pt[:], ps[:], Exp, scale=SCALE)
                else:
                    # diagonal tile: exp the valid columns, then zero the
                    # above-diagonal entries of the in-block diagonal via a
                    # DVE mask-multiply (and the columns left of the valid
                    # range via memset — PV streams the full chunk on its
                    # closing matmul)
                    pt3 = pt[:].rearrange("p (b w) -> p b w", b=2)[:, :, c0:]
                    ps3 = ps[:].rearrange("p (b w) -> p b w", b=2)[:, :, c0:]
                    if r > 0:
                        nc.gpsimd.memset(
                            pt[:].rearrange("p (b w) -> p b w", b=2)[:, :, 0:c0],
                            0.0)
                    nc.scalar.activation(pt3, ps3, Exp, scale=SCALE)
                    for half in range(2):
                        ptd = pt[:, half * 512 + c0:half * 512 + c0 + 128]
                        nc.vector.tensor_mul(ptd, ptd, tri_sb[:])

        def emit_pv(pr, ci):
            # PV: V_aug stationary [128k, 65], P^T moving.
            # Output O^T_aug [65, 512q]: rows 0:64 = O^T, row 64 = sums.
            # Diagonal tiles r in {1,2} stream only valid columns; the last
            # tile streams full width (its masked cols are zero in pt) so
            # every PSUM element's accumulation closes with stop=True.
            # The UNNORMALIZED O^T is cast straight into ot_sb (freeing the
            # po bank as soon as the recip-input ln also reads it);
            # normalization happens in place two pairs later.
            q0 = ci * 512
            njs = 4 * ci + 4
            pts = pts_map.pop((pr, ci))
            pos = []
            # every diagonal tile (r>=1) streams only its valid columns:
            # the j=0 start=True clears has_written for the whole bank, so
            # elements the later partial-width tiles never touch keep their
            # earlier accumulated value, and the closing stop=True only
            # needs to cover its own columns
            for half in range(2):
                h = 2 * pr + half
                fi, row = h // 2, (h % 2) * 64
                po = ps_o.tile([DH + 1, 512], F32, tag="o", name="po")
                pos.append(po)
                for j in range(njs):
                    r = j - 4 * ci
                    c0 = 128 * r if r >= 1 else 0
                    nc.tensor.matmul(
                        po[:, c0:],
                        v_sb[j][:, h * VSTRIDE:h * VSTRIDE + DH + 1],
                        pts[j][:, half * 512 + c0:(half + 1) * 512],
                        start=(j == 0), stop=(j == njs - 1))
                nc.vector.tensor_copy(
                    ot_sb[fi][row:row + 64, q0:q0 + 512], po[0:DH, :])
            return [pr, ci, pos, None]

        def emit_recip(rec):
            # ln then exp(-x) of both sums rows (same ACT table set as the
            # softmax exps). Emitted inside the NEXT pair's exp stream so
            # the ACT never stalls waiting for the PV to finish.
            pr, ci, pos, _ = rec
            lrow = work.tile([1, 1024], F32, tag="lrow", name="lrow", bufs=1)
            rrow = work.tile([1, 1024], BF16, tag="rrow", name="rrow", bufs=2)
            for half in range(2):
                nc.scalar.activation(lrow[0:1, half * 512:(half + 1) * 512],
                                     pos[half][DH:DH + 1, :], Ln)
            nc.scalar.activation(rrow[:], lrow[:], Exp, scale=-1.0)
            rec[2] = None
            rec[3] = rrow

        def norm_finish(rec):
            # rank-1 matmuls broadcast each half's recip row across 64
            # partitions, then DVE scales O^T in place. Runs two pairs
            # after the PV, so the recip rows are always ready.
            pr, ci, _, rrow = rec
            q0 = ci * 512
            for half in range(2):
                h = 2 * pr + half
                fi, row = h // 2, (h % 2) * 64
                rb_ps = ps_mm.tile([DH, 512], F32, tag="mm", name="rb_ps")
                nc.tensor.matmul(
                    rb_ps[:], ones64[:],
                    rrow[0:1, half * 512:(half + 1) * 512],
                    start=True, stop=True)
                ot = ot_sb[fi][row:row + 64, q0:q0 + 512]
                nc.vector.tensor_mul(ot, ot, rb_ps[:])

        os2_box = [None]

        def emit_proj(ci, e0, e1):
            # projection for chunk ci's columns (all pairs' OT normalized).
            # Output cast into 2-ei-wide staging tiles; one batched DMA per
            # ei pair on the (otherwise idle) sync/vector queues so the
            # ACT queue never carries output-DMA issues.
            for ei in range(e0, e1):
                p = ps_mm.tile([128, 512], F32, tag="mm", name="p_proj")
                for fi in range(4):
                    nc.tensor.matmul(
                        p[:], wo_sb[fi][:, ei * 128:(ei + 1) * 128],
                        ot_sb[fi][:, ci * 512:(ci + 1) * 512],
                        start=(fi == 0), stop=(fi == 3))
                if ei % 2 == 0:
                    os2_box[0] = work.tile([128, 1024], BF16, tag="os2",
                                           name="os2", bufs=3)
                os2 = os2_box[0]
                nc.vector.tensor_copy(
                    os2[:, (ei % 2) * 512:(ei % 2 + 1) * 512], p[:])
                if ei % 2 == 1:
                    eng = nc.sync if ei % 4 == 1 else nc.gpsimd
                    eng.dma_start(
                        outT.ap()[(ei - 1) * 128:(ei + 1) * 128,
                                  ci * 512:(ci + 1) * 512]
                            .rearrange("(e p) c -> p e c", e=2),
                        os2[:].rearrange("p (e c) -> p e c", e=2))

        # Two-phase woven schedule balancing PE-heavy projection work
        # against the ACT-bound exp stream. Phase 1: per head-pair pr, its
        # chunks 0..2, with the NEXT pair's Q/K chains woven in (they only
        # use the mm psum tag, so they slot into exp-paced PE bubbles and
        # the next block's sims start without a projection stall); phase
        # 2: the four chunk-3 pairs, PE-filled with V group 3 and the
        # deferred output projections. Within a pair: off-diagonal sims,
        # previous pair's recip (ACT), V fills, pair n-2's norm_finish,
        # projection fill, Q/K weave, diagonal sims, PV + casts.
        order = [(pr, ci) for pr in range(4) for ci in range(3)]
        order += [(pr, 3) for pr in range(4)]
        v_fill = {(0, 0): [0, 1, 2, 3], (0, 1): [4, 5, 6, 7],
                  (0, 2): [8, 9, 10, 11], (3, 1): [12], (3, 2): [13],
                  (0, 3): [14, 15]}
        proj_fill = {(1, 3): 0, (2, 3): 1, (3, 3): 2}
        # later pairs' Q/K projections as per-chunk fill units, deferred
        # as late as their consumers allow so PE filler reaches the
        # otherwise-starved (2,2)/(3,*) blocks (each part only uses the
        # mm psum tag, so no recip flush is needed before it).
        qk_parts = {
            (0, 0): [("q", 0, 1), ("k", 0, 1), ("q", 1, 0), ("k", 1, 0)],
            (0, 1): [("q", 0, 2), ("k", 0, 2), ("q", 1, 1), ("k", 1, 1)],
            (0, 2): [("q", 0, 3), ("k", 0, 3), ("q", 1, 2), ("k", 1, 2)],
            (1, 0): [("q", 2, 0), ("k", 2, 0)],
            (1, 1): [("q", 2, 1), ("k", 2, 1)],
            (1, 2): [("q", 2, 2), ("k", 2, 2), ("q", 1, 3), ("k", 1, 3)],
            (2, 0): [("q", 3, 0), ("k", 3, 0)],
            (2, 1): [("q", 3, 1), ("k", 3, 1)],
            (2, 2): [("q", 3, 2), ("k", 3, 2), ("q", 2, 3), ("k", 2, 3)],
            (3, 0): [("q", 3, 3)],
            (3, 1): [("k", 3, 3)],
        }
        # pair 0's chunk-0 Q/K only — the first sims start once these land;
        # the remaining pair-0 chunks ride the early blocks' fill slots so
        # the in-order PE queue never blocks on late token quarters
        emit_qk_part("q", 0, 0)
        emit_qk_part("k", 0, 0)
        pipe = []   # records awaiting recip (last) / norm_finish (first)
        for pr, ci in order:
            emit_sim(pr, ci, 0, 4 * ci)
            if pipe and pipe[-1][3] is None:
                emit_recip(pipe[-1])
            while len(pipe) >= 2:
                norm_finish(pipe.pop(0))
            if (pr, ci) in proj_fill:
                emit_proj(proj_fill[pr, ci], 0, 8)
            if (pr, ci) != (0, 3):
                emit_sim(pr, ci, 4 * ci, 4 * ci + 4)
            for ti in v_fill.get((pr, ci), []):
                emit_v(ti)
            pipe.append(emit_pv(pr, ci))
            for qn, fi, tck in qk_parts.get((pr, ci), []):
                emit_qk_part(qn, fi, tck)
            if (pr, ci) == (2, 2):
                # prestage (0,3)'s diagonal sims here: their exps run in
                # this block's ACT slack instead of thickening the already
                # exp-bound phase-2 entry (pt tags 12-15 are untouched
                # until (1,3), so single-buffered tiles are safe)
                emit_sim(0, 3, 12, 16)
        emit_recip(pipe[-1])
        norm_finish(pipe.pop(0))
        norm_finish(pipe.pop(0))
        emit_proj(NQC - 1, 0, 8)
    return nc


_NC = None


def _get_nc():
    global _NC
    if _NC is None:
        _patch_walrus_wait_limit()
        _NC = build_kernel()
    return _NC


def _host_tri():
    # S^T orientation: rows = k tokens, cols = q tokens; valid iff q >= k
    return np.triu(np.ones((128, 128), dtype=np.float32)).astype(
        ml_dtypes.bfloat16)


def kernel(x, w_qkv, w_out, _trace=False, _trace_kwargs=None):
    x = np.asarray(x, dtype=np.float32)
    w_qkv = np.asarray(w_qkv, dtype=np.float32)
    w_out = np.asarray(w_out, dtype=np.float32)
    nc = _get_nc()

    tri = _host_tri()
    in_maps = []
    for c in range(NCORES):
        b, g = c // 2, c % 2
        cols = slice(g * FPC, (g + 1) * FPC)
        wq_ = w_qkv[:, 0 * DIM:1 * DIM][:, cols]
        wk_ = w_qkv[:, 1 * DIM:2 * DIM][:, cols]
        wv_ = w_qkv[:, 2 * DIM:3 * DIM][:, cols]
        # xT[di*128+p, tq*512+t] -> host[tq*128+p, di*512+t] so each token
        # quarter is a contiguous [128, 4096] block
        xt = np.ascontiguousarray(
            x[b].T.reshape(8, 128, 4, 512).transpose(2, 1, 0, 3)
                  .reshape(512, 4096))
        in_maps.append({
            "xT": xt.astype(ml_dtypes.bfloat16),
            "wqk": np.concatenate([wq_, wk_], axis=1)
                     .astype(ml_dtypes.bfloat16),
            "wv": np.ascontiguousarray(
                wv_.reshape(8, 128, 512).transpose(1, 0, 2)
                   .reshape(128, 4096)).astype(ml_dtypes.bfloat16),
            "wo": w_out[g * FPC:(g + 1) * FPC, :].astype(ml_dtypes.bfloat16),
            "tri": tri,
        })

    res = run_bass_kernel_spmd(
        nc, in_maps, core_ids=list(range(NCORES)),
        trace=_trace, **(_trace_kwargs or {}))
    out = np.empty((4, SEQ, DIM), dtype=np.float32)
    for b in range(4):
        out[b] = (res.results[2 * b]["outT"].astype(np.float32)
                  + res.results[2 * b + 1]["outT"].astype(np.float32)).T
    if _trace:
        kernel.last_results = res
    return out



# revision 62
# speedup vs baseline: 1.0008x; 1.0008x over previous
"""Causal MHA (batch=4, seq=2048, dim=1024, 16 heads x 64) on 8 TRN2 NeuronCores.

Sharding: core c handles batch b = c//2 and head-group g = c%2 (8 heads).
Each core computes QKV projections for its heads, causal attention, and a
partial output projection over its 512 features. The host sums the two
partial projections per batch and transposes back.

All matmuls run in bf16 (fp32 PSUM accumulate); softmax runs without max
subtraction (logits are bounded ~|8|), with the row sums produced by an
extra ones-column appended to V during the PV matmul. The causal mask on
diagonal S^T blocks is a post-exp DVE multiply by a 0/1 triangle (off
the PE). The recip chain (ln, exp(-x)) runs on ACT; rank-1 matmuls
broadcast the recip rows and DVE normalizes O^T in place two blocks
later.

Scheduling notes (hard-won on HW):
- The chip power-governor downclocks ~20% chip-wide (2.4->2.0GHz) if the
  8 cores pack engines or DMA too densely; v1-level density at full
  clock beats denser schedules. Fine-grained strided DMA (1KB lines)
  also trips it — all transfers use coarse contiguous lines, with xT
  host-preshuffled into token-quarter-major layout.
- Inputs land as per-di [wq|wk|wv] tiles on the two HW rings and
  token-quarter x tiles (SWDGE + rings), so the first QKV chain is
  DMA-paced from ~11us. Q/K live in per-chunk tiles; later pairs'
  projections are emitted as per-chunk fill units, deferred as late as
  consumers allow so PE filler reaches the otherwise-starved
  (2,2)/(3,*) blocks (this halved the HAM re-throttle time).
- Per (head-pair, q-chunk), sims stream at the exp pace via the
  2-buffer sim-PSUM round-robin; PV, V tiles, projection columns and
  the deferred Q/K parts fill the PE between them. Diagonal tiles
  stream only their valid q columns on both the sim and PV matmuls.
- Outputs are cast into 2-row-block staging tiles and DMA'd in batched
  transfers on the sync/SWDGE queues, keeping the ACT queue free of
  DMA issues and the tail drain short.
"""
import sys

sys.path.insert(0, "/opt/trn_rl_repo")

import json
import numpy as np
import ml_dtypes
from contextlib import ExitStack

import concourse.bass as bass
import concourse.tile as tile
from concourse import mybir
from concourse import bass_utils as _bu
from concourse.bass_utils import run_bass_kernel_spmd

LDW_OPT = False  # walrus ldw-opt rejects bass-emitted Ldweights outright

BF16 = mybir.dt.bfloat16
F32 = mybir.dt.float32
F32R = mybir.dt.float32r
Exp = mybir.ActivationFunctionType.Exp
Ln = mybir.ActivationFunctionType.Ln

DIM = 1024
SEQ = 2048
NH = 16          # total heads
HPC = 8          # heads per core
DH = 64          # head dim
SCALE = DH ** -0.5
NCORES = 8
FPC = HPC * DH   # features per core = 512
NKT = SEQ // 128   # 16 k-tiles of 128
NQC = SEQ // 512   # 4 q-chunks of 512
VSTRIDE = DH + 2   # 66: V columns per head incl. ones col + pad

_WALRUS_PATCHED = False


def _patch_walrus_wait_limit():
    """This container's walrus rejects >1 sem wait per instruction
    (CoreV3 setupSyncWait). Tile's tail drain carries one wait per live
    proc; split the extras into preceding single-wait Drain carriers at
    BIR-JSON serialization time."""
    global _WALRUS_PATCHED
    if _WALRUS_PATCHED:
        return
    _WALRUS_PATCHED = True

    if LDW_OPT:
        orig_run = _bu.run_command

        def run_patched(cmd, *a, **k):
            cmd = ["--enable-ldw-opt=true" if c == "--enable-ldw-opt=false" else c
                   for c in cmd]
            return orig_run(cmd, *a, **k)

        _bu.run_command = run_patched

    orig = bass.Bass.to_json_bytes

    def _merge_ldw_halves(insts):
        """Fold row-tiled Ldweights pairs ([64,128] at row 0 + [64,128] at
        row 64 of the same tensor) into one [128,128] load carrying both
        halves' waits."""
        out = []
        pend = None  # (index_in_out, inst) of a candidate row-0 half
        for inst in insts:
            op = inst["opcode"]
            if inst.get("engine") != "PE":
                out.append(inst)
                continue
            if op == "Ldweights" and inst.get("tile_size") == [64, 128]:
                ap = inst["ins"][0].get("ap")
                if inst.get("tile_position") == [0, 0] and ap and ap[0][1] == 64:
                    out.append(inst)
                    pend = (len(out) - 1, inst)
                    continue
                if (pend is not None
                        and inst.get("tile_position") == [64, 0] and ap
                        and ap[0][1] == 64):
                    a = pend[1]
                    aap = a["ins"][0]["ap"]
                    same = (a["ins"][0].get("memref") == inst["ins"][0].get("memref")
                            and aap[0][0] == ap[0][0] and aap[1] == ap[1]
                            and inst["ins"][0].get("offset", 0)
                            == a["ins"][0].get("offset", 0) + 64 * aap[0][0])
                    b_si = inst.get("sync_info") or {}
                    if same and not b_si.get("on_update"):
                        aap[0][1] = 128
                        a["tile_size"] = [128, 128]
                        a.setdefault("sync_info", {"on_update": [], "on_wait": []})
                        a["sync_info"].setdefault("on_wait", [])
                        a["sync_info"]["on_wait"].extend(b_si.get("on_wait") or [])
                        pend = None
                        continue
                out.append(inst)
                pend = None
            else:
                if op not in ("Matmult", "NoOp"):
                    pend = None
                out.append(inst)
        return out

    def patched(self, *a, **k):
        d = json.loads(orig(self, *a, **k))
        for f in d["functions"]:
            for bb in f["blocks"]:
                bb["instructions"] = _merge_ldw_halves(bb["instructions"])
                out = []
                last_ldw = None  # (key, still_valid)
                for inst in bb["instructions"]:
                    si = inst.get("sync_info")
                    ow = (si or {}).get("on_wait") or []
                    op = inst["opcode"]

                    def emit_carriers(waits):
                        for j, w in enumerate(waits):
                            out.append({
                                "name": f"{inst['name']}__w{j}",
                                "opcode": "NoOp",
                                "engine": inst["engine"],
                                "ins": [], "outs": [],
                                "debug": inst.get("debug", 0),
                                "sync_info": {"on_update": [], "on_wait": [w]},
                            })

                    # drop a Ldweights identical to the previous one when only
                    # Matmult/NoOp sit between (weights already resident);
                    # also fold the row-tiled [64,128]+[64,128] half-pair into
                    # the single [128,128] load emitted by _merge_ldw_halves
                    if op == "Ldweights" and inst["engine"] == "PE":
                        key = json.dumps(
                            [inst.get("ins"), inst.get("tile_position"),
                             inst.get("tile_size")], sort_keys=True)
                        if last_ldw == key and not (si or {}).get("on_update"):
                            emit_carriers(ow)
                            continue
                        last_ldw = key
                    elif inst["engine"] == "PE" and op not in ("Matmult", "NoOp"):
                        last_ldw = None

                    if len(ow) > 1:
                        emit_carriers(ow[:-1])
                        si["on_wait"] = [ow[-1]]
                    out.append(inst)
                bb["instructions"] = out
        return json.dumps(d).encode()

    bass.Bass.to_json_bytes = patched


def build_kernel():
    nc = bass.Bass()
    # host packs xT quarter-major: [tq*128+p, di*512+t] so each token
    # quarter is one contiguous [128, 4096] transfer with 4KB lines
    xT = nc.declare_dram_parameter("xT", [4 * 128, 8 * 512], BF16,
                                   isOutput=False)
    # wq|wk per-di rows (the first chains' critical bytes); wv separately,
    # host-packed [p, di*512+c] so it is one contiguous 8KB-line transfer
    wqk = nc.declare_dram_parameter("wqk", [DIM, 2 * FPC], BF16,
                                    isOutput=False)
    wv = nc.declare_dram_parameter("wv", [128, 8 * FPC], BF16,
                                   isOutput=False)
    wo = nc.declare_dram_parameter("wo", [FPC, DIM], BF16, isOutput=False)
    # tri = inclusive lower-triangular 0/1 mask; the diagonal S^T block is
    # exp'd unmasked (logits are bounded, no overflow) and the above-diag
    # entries are zeroed by a DVE multiply, keeping the mask off the PE
    tri = nc.declare_dram_parameter("tri", [128, 128], BF16, isOutput=False)
    outT = nc.declare_dram_parameter("outT", [DIM, SEQ], BF16, isOutput=True)

    with tile.TileContext(nc) as tc, ExitStack() as ctx:
        persist = ctx.enter_context(tc.tile_pool(name="persist", bufs=1))
        work = ctx.enter_context(tc.tile_pool(name="work", bufs=4))
        pt_pool = ctx.enter_context(tc.tile_pool(name="pt", bufs=1))
        ps_mm = ctx.enter_context(tc.tile_pool(name="ps_mm", bufs=2, space="PSUM"))
        ps_s = ctx.enter_context(tc.tile_pool(name="ps_s", bufs=2, space="PSUM"))
        ps_o = ctx.enter_context(tc.tile_pool(name="ps_o", bufs=2, space="PSUM"))

        # ---- load inputs. Fine-grained tiles so consumers start as soon
        # as their own bytes land: weights as per-di [wq|wk|wv] tiles
        # (contiguous 3KB lines) split across the two HW rings, xT as 4
        # host-preshuffled token-quarter tiles (contiguous 4KB lines),
        # Q/K as per-chunk tiles. The first QKV chains + sims are then
        # DMA-paced from ~13us instead of waiting for whole tensors. All
        # transfers keep coarse contiguous lines (fine-grained strided
        # patterns measurably downclock the chip via the power governor).
        w_sb = [persist.tile([128, 2 * FPC], BF16, tag=f"w{di}",
                             name=f"w{di}") for di in range(8)]
        wv_wide = persist.tile([128, 8 * FPC], BF16, tag="wv", name="wv")
        xq_sb = [persist.tile([128, 8 * 512], BF16, tag=f"xq{tq}",
                              name=f"xq{tq}") for tq in range(4)]
        wo_wide = persist.tile([128, 4 * DIM], BF16, tag="wo", name="wo")
        tri_sb = persist.tile([128, 128], BF16, tag="tri", name="tri")
        def wsl(name, di):       # [128,FPC] view of weight block di
            if name == "wv":
                return wv_wide[:, di * FPC:(di + 1) * FPC]
            return w_sb[di][:, 0:FPC] if name == "wq" else w_sb[di][:, FPC:]

        def xsl(di, tq):         # [128,512] token-quarter tq of x block di
            return xq_sb[tq][:, di * 512:(di + 1) * 512]

        def w_dma(eng, di):
            eng.dma_start(w_sb[di][:],
                          wqk.ap()[di * 128:(di + 1) * 128, :])

        def x_dma(eng, tq):
            eng.dma_start(xq_sb[tq][:],
                          xT.ap()[tq * 128:(tq + 1) * 128, :])

        # all weight blocks land by ~24us on the two HW rings; xq0 rides
        # the SWDGE front so the first chain has data ~20us; later token
        # quarters trail (their consumers are fills in later blocks)
        nc.scalar.dma_start(tri_sb[:], tri.ap())
        nc.gpsimd.dma_start(xq_sb[0][:], xT.ap()[0 * 128:1 * 128, :])
        w_dma(nc.sync, 0)
        w_dma(nc.scalar, 1)
        w_dma(nc.sync, 2)
        w_dma(nc.scalar, 3)
        w_dma(nc.sync, 4)
        w_dma(nc.scalar, 5)
        w_dma(nc.sync, 6)
        w_dma(nc.scalar, 7)
        nc.gpsimd.dma_start(wv_wide[:], wv.ap())
        x_dma(nc.scalar, 1)
        nc.gpsimd.dma_start(xq_sb[2][:], xT.ap()[2 * 128:3 * 128, :])
        x_dma(nc.sync, 3)
        nc.gpsimd.dma_start(
            wo_wide[:].rearrange("p (fi c) -> p fi c", fi=4),
            wo.ap().rearrange("(fi p) c -> p fi c", fi=4))
        wo_sb = [wo_wide[:, fi * DIM:(fi + 1) * DIM] for fi in range(4)]
        ones64 = persist.tile([1, DH], BF16, tag="ones64")
        nc.gpsimd.memset(ones64[:], 1.0)

        # ---- stage B: QKV projections -----------------------------------
        qk_sb = {"q": [], "k": []}
        for qn in ("q", "k"):
            for fi in range(4):
                qk_sb[qn].append(
                    [persist.tile([128, 512], BF16, tag=f"{qn}{fi}t{tck}",
                                  name=f"{qn}{fi}t{tck}") for tck in range(4)])
        v_sb = [persist.tile([128, HPC * VSTRIDE], BF16, tag=f"v{ti}",
                             name=f"v{ti}") for ti in range(NKT)]

        def emit_qk_part(qn, fi, tck):
            # one token-chunk of a pair's Q or K projection (fill unit)
            wn = "wq" if qn == "q" else "wk"
            ch = ps_mm.tile([128, 512], F32, tag="mm", name="qkp")
            for di in range(8):
                nc.tensor.matmul(
                    ch[:], wsl(wn, di)[:, fi * 128:(fi + 1) * 128],
                    xsl(di, tck),
                    start=(di == 0), stop=(di == 7))
            nc.vector.tensor_copy(qk_sb[qn][fi][tck][:], ch[:])

        def emit_v(ti):
            # V in [token, feature] layout (xT stationary, wv moving), strided
            # into VSTRIDE-blocks with a ones column per head
            t = v_sb[ti]
            p = ps_mm.tile([128, 512], F32, tag="mm", name="p_v")
            for di in range(8):
                nc.tensor.matmul(
                    p[:], xsl(di, ti // 4)[:, (ti % 4) * 128:(ti % 4 + 1) * 128],
                    wsl("wv", di),
                    start=(di == 0), stop=(di == 7))
            dst = t[:].rearrange("p (h c) -> p h c", h=HPC)[:, :, 0:DH]
            src = p[:].rearrange("p (h c) -> p h c", h=HPC)
            nc.vector.tensor_copy(dst, src)
            nc.gpsimd.memset(
                t[:].rearrange("p (h c) -> p h c", h=HPC)[:, :, DH:DH + 1], 1.0)

        ot_sb = [persist.tile([128, SEQ], BF16, tag=f"ot{fi}", name=f"ot{fi}")
                 for fi in range(4)]
        pts_map = {}

        def emit_sim(pr, ci, j0, j1):
            # S^T strips + exp into pt tiles for (head pair pr, q-chunk ci),
            # k-tiles j0..j1-1. Diagonal tiles (r >= 1) stream only their
            # valid q columns.
            q0 = ci * 512
            pts = pts_map.setdefault((pr, ci), {})
            for j in range(j0, j1):
                r = j - 4 * ci
                c0 = 128 * r if r > 0 else 0   # first valid q col in chunk
                ps = ps_s.tile([128, 1024], F32, tag="s", name="ps_st")
                for half in range(2):   # head A / head B, row-tiled
                    nc.tensor.matmul(
                        ps[:, half * 512 + c0:(half + 1) * 512],
                        qk_sb["k"][pr][j // 4][half * 64:(half + 1) * 64,
                                               (j % 4) * 128:(j % 4 + 1) * 128],
                        qk_sb["q"][pr][ci][half * 64:(half + 1) * 64,
                                           c0:512],
                        start=True, stop=True)
                pt = pt_pool.tile([128, 1024], BF16, tag=f"pt{j}", name="pt",
                                  bufs=2 if j < 14 else 1)
                pts[j] = pt
                if r < 0:
                    nc.scalar.activation(pt[:], ps[:], Exp, scale=SCALE)
                else:
                    # diagonal tile: exp the valid columns, then zero the
                    # above-diagonal entries of the in-block diagonal via a
                    # DVE mask-multiply (and the columns left of the valid
                    # range via memset — PV streams the full chunk on its
                    # closing matmul)
                    pt3 = pt[:].rearrange("p (b w) -> p b w", b=2)[:, :, c0:]
                    ps3 = ps[:].rearrange("p (b w) -> p b w", b=2)[:, :, c0:]
                    if r > 0:
                        nc.gpsimd.memset(
                            pt[:].rearrange("p (b w) -> p b w", b=2)[:, :, 0:c0],
                            0.0)
                    nc.scalar.activation(pt3, ps3, Exp, scale=SCALE)
                    for half in range(2):
                        ptd = pt[:, half * 512 + c0:half * 512 + c0 + 128]
                        nc.vector.tensor_mul(ptd, ptd, tri_sb[:])

        def emit_pv(pr, ci):
            # PV: V_aug stationary [128k, 65], P^T moving.
            # Output O^T_aug [65, 512q]: rows 0:64 = O^T, row 64 = sums.
            # Diagonal tiles r in {1,2} stream only valid columns; the last
            # tile streams full width (its masked cols are zero in pt) so
            # every PSUM element's accumulation closes with stop=True.
            # The UNNORMALIZED O^T is cast straight into ot_sb (freeing the
            # po bank as soon as the recip-input ln also reads it);
            # normalization happens in place two pairs later.
            q0 = ci * 512
            njs = 4 * ci + 4
            pts = pts_map.pop((pr, ci))
            pos = []
            # every diagonal tile (r>=1) streams only its valid columns:
            # the j=0 start=True clears has_written for the whole bank, so
            # elements the later partial-width tiles never touch keep their
            # earlier accumulated value, and the closing stop=True only
            # needs to cover its own columns
            for half in range(2):
                h = 2 * pr + half
                fi, row = h // 2, (h % 2) * 64
                po = ps_o.tile([DH + 1, 512], F32, tag="o", name="po")
                pos.append(po)
                for j in range(njs):
                    r = j - 4 * ci
                    c0 = 128 * r if r >= 1 else 0
                    nc.tensor.matmul(
                        po[:, c0:],
                        v_sb[j][:, h * VSTRIDE:h * VSTRIDE + DH + 1],
                        pts[j][:, half * 512 + c0:(half + 1) * 512],
                        start=(j == 0), stop=(j == njs - 1))
                nc.vector.tensor_copy(
                    ot_sb[fi][row:row + 64, q0:q0 + 512], po[0:DH, :])
            return [pr, ci, pos, None]

        def emit_recip(rec):
            # ln then exp(-x) of both sums rows (same ACT table set as the
            # softmax exps). Emitted inside the NEXT pair's exp stream so
            # the ACT never stalls waiting for the PV to finish.
            pr, ci, pos, _ = rec
            lrow = work.tile([1, 1024], F32, tag="lrow", name="lrow", bufs=1)
            rrow = work.tile([1, 1024], BF16, tag="rrow", name="rrow", bufs=2)
            for half in range(2):
                nc.scalar.activation(lrow[0:1, half * 512:(half + 1) * 512],
                                     pos[half][DH:DH + 1, :], Ln)
            nc.scalar.activation(rrow[:], lrow[:], Exp, scale=-1.0)
            rec[2] = None
            rec[3] = rrow

        def norm_finish(rec):
            # rank-1 matmuls broadcast each half's recip row across 64
            # partitions, then DVE scales O^T in place. Runs two pairs
            # after the PV, so the recip rows are always ready.
            pr, ci, _, rrow = rec
            q0 = ci * 512
            for half in range(2):
                h = 2 * pr + half
                fi, row = h // 2, (h % 2) * 64
                rb_ps = ps_mm.tile([DH, 512], F32, tag="mm", name="rb_ps")
                nc.tensor.matmul(
                    rb_ps[:], ones64[:],
                    rrow[0:1, half * 512:(half + 1) * 512],
                    start=True, stop=True)
                ot = ot_sb[fi][row:row + 64, q0:q0 + 512]
                nc.vector.tensor_mul(ot, ot, rb_ps[:])

        os2_box = [None]

        def emit_proj(ci, e0, e1):
            # projection for chunk ci's columns (all pairs' OT normalized).
            # Output cast into 2-ei-wide staging tiles; one batched DMA per
            # ei pair on the (otherwise idle) sync/vector queues so the
            # ACT queue never carries output-DMA issues.
            for ei in range(e0, e1):
                p = ps_mm.tile([128, 512], F32, tag="mm", name="p_proj")
                for fi in range(4):
                    nc.tensor.matmul(
                        p[:], wo_sb[fi][:, ei * 128:(ei + 1) * 128],
                        ot_sb[fi][:, ci * 512:(ci + 1) * 512],
                        start=(fi == 0), stop=(fi == 3))
                if ei % 2 == 0:
                    os2_box[0] = work.tile([128, 1024], BF16, tag="os2",
                                           name="os2", bufs=3)
                os2 = os2_box[0]
                nc.vector.tensor_copy(
                    os2[:, (ei % 2) * 512:(ei % 2 + 1) * 512], p[:])
                if ei % 2 == 1:
                    eng = nc.sync if ei % 4 == 1 else nc.gpsimd
                    eng.dma_start(
                        outT.ap()[(ei - 1) * 128:(ei + 1) * 128,
                                  ci * 512:(ci + 1) * 512]
                            .rearrange("(e p) c -> p e c", e=2),
                        os2[:].rearrange("p (e c) -> p e c", e=2))

        # Two-phase woven schedule balancing PE-heavy projection work
        # against the ACT-bound exp stream. Phase 1: per head-pair pr, its
        # chunks 0..2, with the NEXT pair's Q/K chains woven in (they only
        # use the mm psum tag, so they slot into exp-paced PE bubbles and
        # the next block's sims start without a projection stall); phase
        # 2: the four chunk-3 pairs, PE-filled with V group 3 and the
        # deferred output projections. Within a pair: off-diagonal sims,
        # previous pair's recip (ACT), V fills, pair n-2's norm_finish,
        # projection fill, Q/K weave, diagonal sims, PV + casts.
        order = [(pr, ci) for pr in range(4) for ci in range(3)]
        order += [(pr, 3) for pr in range(4)]
        v_fill = {(0, 0): [0, 1, 2, 3], (0, 1): [4, 5, 6, 7],
                  (0, 2): [8, 9, 10, 11], (3, 1): [12], (3, 2): [13],
                  (0, 3): [14, 15]}
        proj_fill = {(1, 3): 0, (2, 3): 1, (3, 3): 2}
        # later pairs' Q/K projections as per-chunk fill units, deferred
        # as late as their consumers allow so PE filler reaches the
        # otherwise-starved (2,2)/(3,*) blocks (each part only uses the
        # mm psum tag, so no recip flush is needed before it).
        qk_parts = {
            (0, 0): [("q", 0, 1), ("k", 0, 1), ("q", 1, 0), ("k", 1, 0)],
            (0, 1): [("q", 0, 2), ("k", 0, 2), ("q", 1, 1), ("k", 1, 1)],
            (0, 2): [("q", 0, 3), ("k", 0, 3), ("q", 1, 2), ("k", 1, 2)],
            (1, 0): [("q", 2, 0), ("k", 2, 0)],
            (1, 1): [("q", 2, 1), ("k", 2, 1)],
            (1, 2): [("q", 2, 2), ("k", 2, 2), ("q", 1, 3), ("k", 1, 3)],
            (2, 0): [("q", 3, 0), ("k", 3, 0)],
            (2, 1): [("q", 3, 1), ("k", 3, 1)],
            (2, 2): [("q", 3, 2), ("k", 3, 2), ("q", 2, 3), ("k", 2, 3)],
            (3, 0): [("q", 3, 3)],
            (3, 1): [("k", 3, 3)],
        }
        # pair 0's chunk-0 Q/K only — the first sims start once these land;
        # the remaining pair-0 chunks ride the early blocks' fill slots so
        # the in-order PE queue never blocks on late token quarters
        emit_qk_part("q", 0, 0)
        emit_qk_part("k", 0, 0)
        pipe = []   # records awaiting recip (last) / norm_finish (first)
        for pr, ci in order:
            emit_sim(pr, ci, 0, 4 * ci)
            if pipe and pipe[-1][3] is None:
                emit_recip(pipe[-1])
            while len(pipe) >= 2:
                norm_finish(pipe.pop(0))
            if (pr, ci) in proj_fill:
                emit_proj(proj_fill[pr, ci], 0, 8)
            emit_sim(pr, ci, 4 * ci, 4 * ci + 4)
            for ti in v_fill.get((pr, ci), []):
                emit_v(ti)
            pipe.append(emit_pv(pr, ci))
            for qn, fi, tck in qk_parts.get((pr, ci), []):
                emit_qk_part(qn, fi, tck)
        emit_recip(pipe[-1])
        norm_finish(pipe.pop(0))
        norm_finish(pipe.pop(0))
        emit_proj(NQC - 1, 0, 8)
    return nc


_NC = None


def _get_nc():
    global _NC
    if _NC is None:
        _patch_walrus_wait_limit()
        _NC = build_kernel()
    return _NC


def _host_tri():
    # S^T orientation: rows = k tokens, cols = q tokens; valid iff q >= k
    return np.triu(np.ones((128, 128), dtype=np.float32)).astype(
        ml_dtypes.bfloat16)


def kernel(x, w_qkv, w_out, _trace=False, _trace_kwargs=None):
    x = np.asarray(x, dtype=np.float32)
    w_qkv = np.asarray(w_qkv, dtype=np.float32)
    w_out = np.asarray(w_out, dtype=np.float32)
    nc = _get_nc()

    tri = _host_tri()
    in_maps = []
    for c in range(NCORES):
        b, g = c // 2, c % 2
        cols = slice(g * FPC, (g + 1) * FPC)
        wq_ = w_qkv[:, 0 * DIM:1 * DIM][:, cols]
        wk_ = w_qkv[:, 1 * DIM:2 * DIM][:, cols]
        wv_ = w_qkv[:, 2 * DIM:3 * DIM][:, cols]
        # xT[di*128+p, tq*512+t] -> host[tq*128+p, di*512+t] so each token
        # quarter is a contiguous [128, 4096] block
        xt = np.ascontiguousarray(
            x[b].T.reshape(8, 128, 4, 512).transpose(2, 1, 0, 3)
                  .reshape(512, 4096))
        in_maps.append({
            "xT": xt.astype(ml_dtypes.bfloat16),
            "wqk": np.concatenate([wq_, wk_], axis=1)
                     .astype(ml_dtypes.bfloat16),
            "wv": np.ascontiguousarray(
                wv_.reshape(8, 128, 512).transpose(1, 0, 2)
                   .reshape(128, 4096)).astype(ml_dtypes.bfloat16),
            "wo": w_out[g * FPC:(g + 1) * FPC, :].astype(ml_dtypes.bfloat16),
            "tri": tri,
        })

    res = run_bass_kernel_spmd(
        nc, in_maps, core_ids=list(range(NCORES)),
        trace=_trace, **(_trace_kwargs or {}))
    out = np.empty((4, SEQ, DIM), dtype=np.float32)
    for b in range(4):
        out[b] = (res.results[2 * b]["outT"].astype(np.float32)
                  + res.results[2 * b + 1]["outT"].astype(np.float32)).T
    if _trace:
        kernel.last_results = res
    return out



# revision 63
# speedup vs baseline: 1.0076x; 1.0068x over previous
"""Causal MHA (batch=4, seq=2048, dim=1024, 16 heads x 64) on 8 TRN2 NeuronCores.

Sharding: core c handles batch b = c//2 and head-group g = c%2 (8 heads).
Each core computes QKV projections for its heads, causal attention, and a
partial output projection over its 512 features. The host sums the two
partial projections per batch and transposes back.

All matmuls run in bf16 (fp32 PSUM accumulate); softmax runs without max
subtraction (logits are bounded ~|8|), with the row sums produced by an
extra ones-column appended to V during the PV matmul. The causal mask on
diagonal S^T blocks is a post-exp DVE multiply by a 0/1 triangle (off
the PE). The recip chain (ln, exp(-x)) runs on ACT; rank-1 matmuls
broadcast the recip rows and DVE normalizes O^T in place two blocks
later.

Scheduling notes (hard-won on HW):
- The chip power-governor downclocks ~20% chip-wide (2.4->2.0GHz) if the
  8 cores pack engines or DMA too densely; v1-level density at full
  clock beats denser schedules. Fine-grained strided DMA (1KB lines)
  also trips it — all transfers use coarse contiguous lines, with xT
  host-preshuffled into token-quarter-major layout.
- Inputs land as per-di [wq|wk|wv] tiles on the two HW rings and
  token-quarter x tiles (SWDGE + rings), so the first QKV chain is
  DMA-paced from ~11us. Q/K live in per-chunk tiles; later pairs'
  projections are emitted as per-chunk fill units, deferred as late as
  consumers allow so PE filler reaches the otherwise-starved
  (2,2)/(3,*) blocks (this halved the HAM re-throttle time).
- Per (head-pair, q-chunk), sims stream at the exp pace via the
  2-buffer sim-PSUM round-robin; PV, V tiles, projection columns and
  the deferred Q/K parts fill the PE between them. Diagonal tiles
  stream only their valid q columns on both the sim and PV matmuls.
- Outputs are cast into 2-row-block staging tiles and DMA'd in batched
  transfers on the sync/SWDGE queues, keeping the ACT queue free of
  DMA issues and the tail drain short.
"""
import sys

sys.path.insert(0, "/opt/trn_rl_repo")

import json
import numpy as np
import ml_dtypes
from contextlib import ExitStack

import concourse.bass as bass
import concourse.tile as tile
from concourse import mybir
from concourse import bass_utils as _bu
from concourse.bass_utils import run_bass_kernel_spmd

LDW_OPT = False  # walrus ldw-opt rejects bass-emitted Ldweights outright

BF16 = mybir.dt.bfloat16
F32 = mybir.dt.float32
F32R = mybir.dt.float32r
Exp = mybir.ActivationFunctionType.Exp
Ln = mybir.ActivationFunctionType.Ln

DIM = 1024
SEQ = 2048
NH = 16          # total heads
HPC = 8          # heads per core
DH = 64          # head dim
SCALE = DH ** -0.5
NCORES = 8
FPC = HPC * DH   # features per core = 512
NKT = SEQ // 128   # 16 k-tiles of 128
NQC = SEQ // 512   # 4 q-chunks of 512
VSTRIDE = DH + 2   # 66: V columns per head incl. ones col + pad

_WALRUS_PATCHED = False


def _patch_walrus_wait_limit():
    """This container's walrus rejects >1 sem wait per instruction
    (CoreV3 setupSyncWait). Tile's tail drain carries one wait per live
    proc; split the extras into preceding single-wait Drain carriers at
    BIR-JSON serialization time."""
    global _WALRUS_PATCHED
    if _WALRUS_PATCHED:
        return
    _WALRUS_PATCHED = True

    if LDW_OPT:
        orig_run = _bu.run_command

        def run_patched(cmd, *a, **k):
            cmd = ["--enable-ldw-opt=true" if c == "--enable-ldw-opt=false" else c
                   for c in cmd]
            return orig_run(cmd, *a, **k)

        _bu.run_command = run_patched

    orig = bass.Bass.to_json_bytes

    def _merge_ldw_halves(insts):
        """Fold row-tiled Ldweights pairs ([64,128] at row 0 + [64,128] at
        row 64 of the same tensor) into one [128,128] load carrying both
        halves' waits."""
        out = []
        pend = None  # (index_in_out, inst) of a candidate row-0 half
        for inst in insts:
            op = inst["opcode"]
            if inst.get("engine") != "PE":
                out.append(inst)
                continue
            if op == "Ldweights" and inst.get("tile_size") == [64, 128]:
                ap = inst["ins"][0].get("ap")
                if inst.get("tile_position") == [0, 0] and ap and ap[0][1] == 64:
                    out.append(inst)
                    pend = (len(out) - 1, inst)
                    continue
                if (pend is not None
                        and inst.get("tile_position") == [64, 0] and ap
                        and ap[0][1] == 64):
                    a = pend[1]
                    aap = a["ins"][0]["ap"]
                    same = (a["ins"][0].get("memref") == inst["ins"][0].get("memref")
                            and aap[0][0] == ap[0][0] and aap[1] == ap[1]
                            and inst["ins"][0].get("offset", 0)
                            == a["ins"][0].get("offset", 0) + 64 * aap[0][0])
                    b_si = inst.get("sync_info") or {}
                    if same and not b_si.get("on_update"):
                        aap[0][1] = 128
                        a["tile_size"] = [128, 128]
                        a.setdefault("sync_info", {"on_update": [], "on_wait": []})
                        a["sync_info"].setdefault("on_wait", [])
                        a["sync_info"]["on_wait"].extend(b_si.get("on_wait") or [])
                        pend = None
                        continue
                out.append(inst)
                pend = None
            else:
                if op not in ("Matmult", "NoOp"):
                    pend = None
                out.append(inst)
        return out

    def patched(self, *a, **k):
        d = json.loads(orig(self, *a, **k))
        for f in d["functions"]:
            for bb in f["blocks"]:
                bb["instructions"] = _merge_ldw_halves(bb["instructions"])
                out = []
                last_ldw = None  # (key, still_valid)
                for inst in bb["instructions"]:
                    si = inst.get("sync_info")
                    ow = (si or {}).get("on_wait") or []
                    op = inst["opcode"]

                    def emit_carriers(waits):
                        for j, w in enumerate(waits):
                            out.append({
                                "name": f"{inst['name']}__w{j}",
                                "opcode": "NoOp",
                                "engine": inst["engine"],
                                "ins": [], "outs": [],
                                "debug": inst.get("debug", 0),
                                "sync_info": {"on_update": [], "on_wait": [w]},
                            })

                    # drop a Ldweights identical to the previous one when only
                    # Matmult/NoOp sit between (weights already resident);
                    # also fold the row-tiled [64,128]+[64,128] half-pair into
                    # the single [128,128] load emitted by _merge_ldw_halves
                    if op == "Ldweights" and inst["engine"] == "PE":
                        key = json.dumps(
                            [inst.get("ins"), inst.get("tile_position"),
                             inst.get("tile_size")], sort_keys=True)
                        if last_ldw == key and not (si or {}).get("on_update"):
                            emit_carriers(ow)
                            continue
                        last_ldw = key
                    elif inst["engine"] == "PE" and op not in ("Matmult", "NoOp"):
                        last_ldw = None

                    if len(ow) > 1:
                        emit_carriers(ow[:-1])
                        si["on_wait"] = [ow[-1]]
                    out.append(inst)
                bb["instructions"] = out
        return json.dumps(d).encode()

    bass.Bass.to_json_bytes = patched


def build_kernel():
    nc = bass.Bass()
    # host packs xT quarter-major: [tq*128+p, di*512+t] so each token
    # quarter is one contiguous [128, 4096] transfer with 4KB lines
    xT = nc.declare_dram_parameter("xT", [4 * 128, 8 * 512], BF16,
                                   isOutput=False)
    # wq|wk per-di rows (the first chains' critical bytes); wv separately,
    # host-packed [p, di*512+c] so it is one contiguous 8KB-line transfer
    wqk = nc.declare_dram_parameter("wqk", [DIM, 2 * FPC], BF16,
                                    isOutput=False)
    wv = nc.declare_dram_parameter("wv", [128, 8 * FPC], BF16,
                                   isOutput=False)
    wo = nc.declare_dram_parameter("wo", [FPC, DIM], BF16, isOutput=False)
    # tri = inclusive lower-triangular 0/1 mask; the diagonal S^T block is
    # exp'd unmasked (logits are bounded, no overflow) and the above-diag
    # entries are zeroed by a DVE multiply, keeping the mask off the PE
    tri = nc.declare_dram_parameter("tri", [128, 128], BF16, isOutput=False)
    outT = nc.declare_dram_parameter("outT", [DIM, SEQ], BF16, isOutput=True)

    with tile.TileContext(nc) as tc, ExitStack() as ctx:
        persist = ctx.enter_context(tc.tile_pool(name="persist", bufs=1))
        work = ctx.enter_context(tc.tile_pool(name="work", bufs=4))
        pt_pool = ctx.enter_context(tc.tile_pool(name="pt", bufs=1))
        ps_mm = ctx.enter_context(tc.tile_pool(name="ps_mm", bufs=2, space="PSUM"))
        ps_s = ctx.enter_context(tc.tile_pool(name="ps_s", bufs=2, space="PSUM"))
        ps_o = ctx.enter_context(tc.tile_pool(name="ps_o", bufs=2, space="PSUM"))

        # ---- load inputs. Fine-grained tiles so consumers start as soon
        # as their own bytes land: weights as per-di [wq|wk|wv] tiles
        # (contiguous 3KB lines) split across the two HW rings, xT as 4
        # host-preshuffled token-quarter tiles (contiguous 4KB lines),
        # Q/K as per-chunk tiles. The first QKV chains + sims are then
        # DMA-paced from ~13us instead of waiting for whole tensors. All
        # transfers keep coarse contiguous lines (fine-grained strided
        # patterns measurably downclock the chip via the power governor).
        w_sb = [persist.tile([128, 2 * FPC], BF16, tag=f"w{di}",
                             name=f"w{di}") for di in range(8)]
        wv_wide = persist.tile([128, 8 * FPC], BF16, tag="wv", name="wv")
        xq_sb = [persist.tile([128, 8 * 512], BF16, tag=f"xq{tq}",
                              name=f"xq{tq}") for tq in range(4)]
        wo_wide = persist.tile([128, 4 * DIM], BF16, tag="wo", name="wo")
        tri_sb = persist.tile([128, 128], BF16, tag="tri", name="tri")
        def wsl(name, di):       # [128,FPC] view of weight block di
            if name == "wv":
                return wv_wide[:, di * FPC:(di + 1) * FPC]
            return w_sb[di][:, 0:FPC] if name == "wq" else w_sb[di][:, FPC:]

        def xsl(di, tq):         # [128,512] token-quarter tq of x block di
            return xq_sb[tq][:, di * 512:(di + 1) * 512]

        def w_dma(eng, di):
            eng.dma_start(w_sb[di][:],
                          wqk.ap()[di * 128:(di + 1) * 128, :])

        def x_dma(eng, tq):
            eng.dma_start(xq_sb[tq][:],
                          xT.ap()[tq * 128:(tq + 1) * 128, :])

        # all weight blocks land by ~24us on the two HW rings; xq0 rides
        # the SWDGE front so the first chain has data ~20us; later token
        # quarters trail (their consumers are fills in later blocks)
        nc.scalar.dma_start(tri_sb[:], tri.ap())
        nc.gpsimd.dma_start(xq_sb[0][:], xT.ap()[0 * 128:1 * 128, :])
        w_dma(nc.sync, 0)
        w_dma(nc.scalar, 1)
        w_dma(nc.sync, 2)
        w_dma(nc.scalar, 3)
        w_dma(nc.sync, 4)
        w_dma(nc.scalar, 5)
        w_dma(nc.sync, 6)
        w_dma(nc.scalar, 7)
        nc.gpsimd.dma_start(wv_wide[:], wv.ap())
        x_dma(nc.scalar, 1)
        nc.gpsimd.dma_start(xq_sb[2][:], xT.ap()[2 * 128:3 * 128, :])
        x_dma(nc.sync, 3)
        nc.gpsimd.dma_start(
            wo_wide[:].rearrange("p (fi c) -> p fi c", fi=4),
            wo.ap().rearrange("(fi p) c -> p fi c", fi=4))
        wo_sb = [wo_wide[:, fi * DIM:(fi + 1) * DIM] for fi in range(4)]
        ones64 = persist.tile([1, DH], BF16, tag="ones64")
        nc.gpsimd.memset(ones64[:], 1.0)

        # ---- stage B: QKV projections -----------------------------------
        qk_sb = {"q": [], "k": []}
        for qn in ("q", "k"):
            for fi in range(4):
                qk_sb[qn].append(
                    [persist.tile([128, 512], BF16, tag=f"{qn}{fi}t{tck}",
                                  name=f"{qn}{fi}t{tck}") for tck in range(4)])
        v_sb = [persist.tile([128, HPC * VSTRIDE], BF16, tag=f"v{ti}",
                             name=f"v{ti}") for ti in range(NKT)]

        def emit_qk_part(qn, fi, tck):
            # one token-chunk of a pair's Q or K projection (fill unit)
            wn = "wq" if qn == "q" else "wk"
            ch = ps_mm.tile([128, 512], F32, tag="mm", name="qkp")
            for di in range(8):
                nc.tensor.matmul(
                    ch[:], wsl(wn, di)[:, fi * 128:(fi + 1) * 128],
                    xsl(di, tck),
                    start=(di == 0), stop=(di == 7))
            nc.vector.tensor_copy(qk_sb[qn][fi][tck][:], ch[:])

        def emit_v(ti):
            # V in [token, feature] layout (xT stationary, wv moving), strided
            # into VSTRIDE-blocks with a ones column per head
            t = v_sb[ti]
            p = ps_mm.tile([128, 512], F32, tag="mm", name="p_v")
            for di in range(8):
                nc.tensor.matmul(
                    p[:], xsl(di, ti // 4)[:, (ti % 4) * 128:(ti % 4 + 1) * 128],
                    wsl("wv", di),
                    start=(di == 0), stop=(di == 7))
            dst = t[:].rearrange("p (h c) -> p h c", h=HPC)[:, :, 0:DH]
            src = p[:].rearrange("p (h c) -> p h c", h=HPC)
            nc.vector.tensor_copy(dst, src)
            nc.gpsimd.memset(
                t[:].rearrange("p (h c) -> p h c", h=HPC)[:, :, DH:DH + 1], 1.0)

        ot_sb = [persist.tile([128, SEQ], BF16, tag=f"ot{fi}", name=f"ot{fi}")
                 for fi in range(4)]
        pts_map = {}

        def emit_sim(pr, ci, j0, j1):
            # S^T strips + exp into pt tiles for (head pair pr, q-chunk ci),
            # k-tiles j0..j1-1. Diagonal tiles (r >= 1) stream only their
            # valid q columns.
            q0 = ci * 512
            pts = pts_map.setdefault((pr, ci), {})
            for j in range(j0, j1):
                r = j - 4 * ci
                c0 = 128 * r if r > 0 else 0   # first valid q col in chunk
                ps = ps_s.tile([128, 1024], F32, tag="s", name="ps_st")
                for half in range(2):   # head A / head B, row-tiled
                    nc.tensor.matmul(
                        ps[:, half * 512 + c0:(half + 1) * 512],
                        qk_sb["k"][pr][j // 4][half * 64:(half + 1) * 64,
                                               (j % 4) * 128:(j % 4 + 1) * 128],
                        qk_sb["q"][pr][ci][half * 64:(half + 1) * 64,
                                           c0:512],
                        start=True, stop=True)
                pt = pt_pool.tile([128, 1024], BF16, tag=f"pt{j}", name="pt",
                                  bufs=2 if j < 14 else 1)
                pts[j] = pt
                if r < 0:
                    nc.scalar.activation(pt[:], ps[:], Exp, scale=SCALE)
                else:
                    # diagonal tile: exp the valid columns, then zero the
                    # above-diagonal entries of the in-block diagonal via a
                    # DVE mask-multiply (and the columns left of the valid
                    # range via memset — PV streams the full chunk on its
                    # closing matmul)
                    pt3 = pt[:].rearrange("p (b w) -> p b w", b=2)[:, :, c0:]
                    ps3 = ps[:].rearrange("p (b w) -> p b w", b=2)[:, :, c0:]
                    if r > 0:
                        nc.gpsimd.memset(
                            pt[:].rearrange("p (b w) -> p b w", b=2)[:, :, 0:c0],
                            0.0)
                    nc.scalar.activation(pt3, ps3, Exp, scale=SCALE)
                    for half in range(2):
                        ptd = pt[:, half * 512 + c0:half * 512 + c0 + 128]
                        nc.vector.tensor_mul(ptd, ptd, tri_sb[:])

        def emit_pv(pr, ci):
            # PV: V_aug stationary [128k, 65], P^T moving.
            # Output O^T_aug [65, 512q]: rows 0:64 = O^T, row 64 = sums.
            # Diagonal tiles r in {1,2} stream only valid columns; the last
            # tile streams full width (its masked cols are zero in pt) so
            # every PSUM element's accumulation closes with stop=True.
            # The UNNORMALIZED O^T is cast straight into ot_sb (freeing the
            # po bank as soon as the recip-input ln also reads it);
            # normalization happens in place two pairs later.
            q0 = ci * 512
            njs = 4 * ci + 4
            pts = pts_map.pop((pr, ci))
            pos = []
            # every diagonal tile (r>=1) streams only its valid columns:
            # the j=0 start=True clears has_written for the whole bank, so
            # elements the later partial-width tiles never touch keep their
            # earlier accumulated value, and the closing stop=True only
            # needs to cover its own columns
            for half in range(2):
                h = 2 * pr + half
                fi, row = h // 2, (h % 2) * 64
                po = ps_o.tile([DH + 1, 512], F32, tag="o", name="po")
                pos.append(po)
                for j in range(njs):
                    r = j - 4 * ci
                    c0 = 128 * r if r >= 1 else 0
                    nc.tensor.matmul(
                        po[:, c0:],
                        v_sb[j][:, h * VSTRIDE:h * VSTRIDE + DH + 1],
                        pts[j][:, half * 512 + c0:(half + 1) * 512],
                        start=(j == 0), stop=(j == njs - 1))
                nc.vector.tensor_copy(
                    ot_sb[fi][row:row + 64, q0:q0 + 512], po[0:DH, :])
            return [pr, ci, pos, None]

        def emit_recip(rec):
            # ln then exp(-x) of both sums rows (same ACT table set as the
            # softmax exps). Emitted inside the NEXT pair's exp stream so
            # the ACT never stalls waiting for the PV to finish.
            pr, ci, pos, _ = rec
            lrow = work.tile([1, 1024], F32, tag="lrow", name="lrow", bufs=1)
            rrow = work.tile([1, 1024], BF16, tag="rrow", name="rrow", bufs=2)
            for half in range(2):
                nc.scalar.activation(lrow[0:1, half * 512:(half + 1) * 512],
                                     pos[half][DH:DH + 1, :], Ln)
            nc.scalar.activation(rrow[:], lrow[:], Exp, scale=-1.0)
            rec[2] = None
            rec[3] = rrow

        def norm_finish(rec):
            # rank-1 matmuls broadcast each half's recip row across 64
            # partitions, then DVE scales O^T in place. Runs two pairs
            # after the PV, so the recip rows are always ready.
            pr, ci, _, rrow = rec
            q0 = ci * 512
            for half in range(2):
                h = 2 * pr + half
                fi, row = h // 2, (h % 2) * 64
                rb_ps = ps_mm.tile([DH, 512], F32, tag="mm", name="rb_ps")
                nc.tensor.matmul(
                    rb_ps[:], ones64[:],
                    rrow[0:1, half * 512:(half + 1) * 512],
                    start=True, stop=True)
                ot = ot_sb[fi][row:row + 64, q0:q0 + 512]
                nc.vector.tensor_mul(ot, ot, rb_ps[:])

        os2_box = [None]

        def emit_proj(ci, e0, e1):
            # projection for chunk ci's columns (all pairs' OT normalized).
            # Output cast into 2-ei-wide staging tiles; one batched DMA per
            # ei pair on the (otherwise idle) sync/vector queues so the
            # ACT queue never carries output-DMA issues.
            for ei in range(e0, e1):
                p = ps_mm.tile([128, 512], F32, tag="mm", name="p_proj")
                for fi in range(4):
                    nc.tensor.matmul(
                        p[:], wo_sb[fi][:, ei * 128:(ei + 1) * 128],
                        ot_sb[fi][:, ci * 512:(ci + 1) * 512],
                        start=(fi == 0), stop=(fi == 3))
                if ei % 2 == 0:
                    os2_box[0] = work.tile([128, 1024], BF16, tag="os2",
                                           name="os2", bufs=3)
                os2 = os2_box[0]
                nc.vector.tensor_copy(
                    os2[:, (ei % 2) * 512:(ei % 2 + 1) * 512], p[:])
                if ei % 2 == 1:
                    eng = nc.sync if ei % 4 == 1 else nc.gpsimd
                    eng.dma_start(
                        outT.ap()[(ei - 1) * 128:(ei + 1) * 128,
                                  ci * 512:(ci + 1) * 512]
                            .rearrange("(e p) c -> p e c", e=2),
                        os2[:].rearrange("p (e c) -> p e c", e=2))

        # Two-phase woven schedule balancing PE-heavy projection work
        # against the ACT-bound exp stream. Phase 1: per head-pair pr, its
        # chunks 0..2, with the NEXT pair's Q/K chains woven in (they only
        # use the mm psum tag, so they slot into exp-paced PE bubbles and
        # the next block's sims start without a projection stall); phase
        # 2: the four chunk-3 pairs, PE-filled with V group 3 and the
        # deferred output projections. Within a pair: off-diagonal sims,
        # previous pair's recip (ACT), V fills, pair n-2's norm_finish,
        # projection fill, Q/K weave, diagonal sims, PV + casts.
        # chunk-major block order: every block gets a uniform filler diet
        # (two deferred Q/K chunk-parts + one V tile), the DMA ramp gets
        # all four pairs' chunk-0 parts (they only need xq0 + wqk), and
        # the chunk projections spread mid-schedule instead of bunching
        # in the exp-bound endgame.
        order = [(pr, ci) for ci in range(4) for pr in range(4)]
        v_fill = {(0, 0): [0, 1, 2, 3], (1, 0): [4], (2, 0): [5],
                  (3, 0): [6], (0, 1): [7], (1, 1): [8], (2, 1): [9],
                  (3, 1): [10], (0, 2): [11], (1, 2): [12], (2, 2): [13],
                  (3, 2): [14], (0, 3): [15]}
        proj_fill = {(1, 1): 0, (1, 2): 1, (1, 3): 2}
        qk_parts = {
            (0, 0): [("q", 1, 0), ("k", 1, 0)],
            (1, 0): [("q", 2, 0), ("k", 2, 0)],
            (2, 0): [("q", 3, 0), ("k", 3, 0)],
            (3, 0): [("q", 0, 1), ("k", 0, 1)],
            (0, 1): [("q", 1, 1), ("k", 1, 1)],
            (1, 1): [("q", 2, 1), ("k", 2, 1)],
            (2, 1): [("q", 3, 1), ("k", 3, 1)],
            (3, 1): [("q", 0, 2), ("k", 0, 2)],
            (0, 2): [("q", 1, 2), ("k", 1, 2)],
            (1, 2): [("q", 2, 2), ("k", 2, 2)],
            (2, 2): [("q", 3, 2), ("k", 3, 2)],
            (3, 2): [("q", 0, 3), ("k", 0, 3)],
            (0, 3): [("q", 1, 3), ("k", 1, 3)],
            (1, 3): [("q", 2, 3), ("k", 2, 3)],
            (2, 3): [("q", 3, 3), ("k", 3, 3)],
        }
        # pair 0's chunk-0 Q/K only — the first sims start once these land;
        # the remaining pair-0 chunks ride the early blocks' fill slots so
        # the in-order PE queue never blocks on late token quarters
        emit_qk_part("q", 0, 0)
        emit_qk_part("k", 0, 0)
        pipe = []   # records awaiting recip (last) / norm_finish (first)
        for pr, ci in order:
            emit_sim(pr, ci, 0, 4 * ci)
            if pipe and pipe[-1][3] is None:
                emit_recip(pipe[-1])
            while len(pipe) >= 2:
                norm_finish(pipe.pop(0))
            if (pr, ci) in proj_fill:
                emit_proj(proj_fill[pr, ci], 0, 8)
            emit_sim(pr, ci, 4 * ci, 4 * ci + 4)
            for ti in v_fill.get((pr, ci), []):
                emit_v(ti)
            pipe.append(emit_pv(pr, ci))
            for qn, fi, tck in qk_parts.get((pr, ci), []):
                emit_qk_part(qn, fi, tck)
        emit_recip(pipe[-1])
        norm_finish(pipe.pop(0))
        norm_finish(pipe.pop(0))
        emit_proj(NQC - 1, 0, 8)
    return nc


_NC = None


def _get_nc():
    global _NC
    if _NC is None:
        _patch_walrus_wait_limit()
        _NC = build_kernel()
    return _NC


def _host_tri():
    # S^T orientation: rows = k tokens, cols = q tokens; valid iff q >= k
    return np.triu(np.ones((128, 128), dtype=np.float32)).astype(
        ml_dtypes.bfloat16)


def kernel(x, w_qkv, w_out, _trace=False, _trace_kwargs=None):
    x = np.asarray(x, dtype=np.float32)
    w_qkv = np.asarray(w_qkv, dtype=np.float32)
    w_out = np.asarray(w_out, dtype=np.float32)
    nc = _get_nc()

    tri = _host_tri()
    in_maps = []
    for c in range(NCORES):
        b, g = c // 2, c % 2
        cols = slice(g * FPC, (g + 1) * FPC)
        wq_ = w_qkv[:, 0 * DIM:1 * DIM][:, cols]
        wk_ = w_qkv[:, 1 * DIM:2 * DIM][:, cols]
        wv_ = w_qkv[:, 2 * DIM:3 * DIM][:, cols]
        # xT[di*128+p, tq*512+t] -> host[tq*128+p, di*512+t] so each token
        # quarter is a contiguous [128, 4096] block
        xt = np.ascontiguousarray(
            x[b].T.reshape(8, 128, 4, 512).transpose(2, 1, 0, 3)
                  .reshape(512, 4096))
        in_maps.append({
            "xT": xt.astype(ml_dtypes.bfloat16),
            "wqk": np.concatenate([wq_, wk_], axis=1)
                     .astype(ml_dtypes.bfloat16),
            "wv": np.ascontiguousarray(
                wv_.reshape(8, 128, 512).transpose(1, 0, 2)
                   .reshape(128, 4096)).astype(ml_dtypes.bfloat16),
            "wo": w_out[g * FPC:(g + 1) * FPC, :].astype(ml_dtypes.bfloat16),
            "tri": tri,
        })

    res = run_bass_kernel_spmd(
        nc, in_maps, core_ids=list(range(NCORES)),
        trace=_trace, **(_trace_kwargs or {}))
    out = np.empty((4, SEQ, DIM), dtype=np.float32)
    for b in range(4):
        out[b] = (res.results[2 * b]["outT"].astype(np.float32)
                  + res.results[2 * b + 1]["outT"].astype(np.float32)).T
    if _trace:
        kernel.last_results = res
    return out



# revision 64
# speedup vs baseline: 1.0311x; 1.0233x over previous
"""Causal MHA (batch=4, seq=2048, dim=1024, 16 heads x 64) on 8 TRN2 NeuronCores.

Sharding: core c handles batch b = c//2 and head-group g = c%2 (8 heads).
Each core computes QKV projections for its heads, causal attention, and a
partial output projection over its 512 features. The host sums the two
partial projections per batch and transposes back.

All matmuls run in bf16 (fp32 PSUM accumulate); softmax runs without max
subtraction (logits are bounded ~|8|), with the row sums produced by an
extra ones-column appended to V during the PV matmul. The causal mask on
diagonal S^T blocks is a post-exp DVE multiply by a 0/1 triangle (off
the PE). The recip chain (ln, exp(-x)) runs on ACT; rank-1 matmuls
broadcast the recip rows and DVE normalizes O^T in place two blocks
later.

Scheduling notes (hard-won on HW):
- The chip power-governor downclocks ~20% chip-wide (2.4->2.0GHz) if the
  8 cores pack engines or DMA too densely; v1-level density at full
  clock beats denser schedules. Fine-grained strided DMA (1KB lines)
  also trips it — all transfers use coarse contiguous lines, with xT
  host-preshuffled into token-quarter-major layout.
- Inputs land as per-di [wq|wk|wv] tiles on the two HW rings and
  token-quarter x tiles (SWDGE + rings), so the first QKV chain is
  DMA-paced from ~11us. Q/K live in per-chunk tiles; later pairs'
  projections are emitted as per-chunk fill units, deferred as late as
  consumers allow so PE filler reaches the otherwise-starved
  (2,2)/(3,*) blocks (this halved the HAM re-throttle time).
- Per (head-pair, q-chunk), sims stream at the exp pace via the
  2-buffer sim-PSUM round-robin; PV, V tiles, projection columns and
  the deferred Q/K parts fill the PE between them. Diagonal tiles
  stream only their valid q columns on both the sim and PV matmuls.
- Outputs are cast into 2-row-block staging tiles and DMA'd in batched
  transfers on the sync/SWDGE queues, keeping the ACT queue free of
  DMA issues and the tail drain short.
"""
import sys

sys.path.insert(0, "/opt/trn_rl_repo")

import json
import numpy as np
import ml_dtypes
from contextlib import ExitStack

import concourse.bass as bass
import concourse.tile as tile
from concourse import mybir
from concourse import bass_utils as _bu
from concourse.bass_utils import run_bass_kernel_spmd

LDW_OPT = False  # walrus ldw-opt rejects bass-emitted Ldweights outright

BF16 = mybir.dt.bfloat16
F32 = mybir.dt.float32
F32R = mybir.dt.float32r
Exp = mybir.ActivationFunctionType.Exp
Ln = mybir.ActivationFunctionType.Ln

DIM = 1024
SEQ = 2048
NH = 16          # total heads
HPC = 8          # heads per core
DH = 64          # head dim
SCALE = DH ** -0.5
NCORES = 8
FPC = HPC * DH   # features per core = 512
NKT = SEQ // 128   # 16 k-tiles of 128
NQC = SEQ // 512   # 4 q-chunks of 512
VSTRIDE = DH + 2   # 66: V columns per head incl. ones col + pad

_WALRUS_PATCHED = False


def _patch_walrus_wait_limit():
    """This container's walrus rejects >1 sem wait per instruction
    (CoreV3 setupSyncWait). Tile's tail drain carries one wait per live
    proc; split the extras into preceding single-wait Drain carriers at
    BIR-JSON serialization time."""
    global _WALRUS_PATCHED
    if _WALRUS_PATCHED:
        return
    _WALRUS_PATCHED = True

    if LDW_OPT:
        orig_run = _bu.run_command

        def run_patched(cmd, *a, **k):
            cmd = ["--enable-ldw-opt=true" if c == "--enable-ldw-opt=false" else c
                   for c in cmd]
            return orig_run(cmd, *a, **k)

        _bu.run_command = run_patched

    orig = bass.Bass.to_json_bytes

    def _merge_ldw_halves(insts):
        """Fold row-tiled Ldweights pairs ([64,128] at row 0 + [64,128] at
        row 64 of the same tensor) into one [128,128] load carrying both
        halves' waits."""
        out = []
        pend = None  # (index_in_out, inst) of a candidate row-0 half
        for inst in insts:
            op = inst["opcode"]
            if inst.get("engine") != "PE":
                out.append(inst)
                continue
            if op == "Ldweights" and inst.get("tile_size") == [64, 128]:
                ap = inst["ins"][0].get("ap")
                if inst.get("tile_position") == [0, 0] and ap and ap[0][1] == 64:
                    out.append(inst)
                    pend = (len(out) - 1, inst)
                    continue
                if (pend is not None
                        and inst.get("tile_position") == [64, 0] and ap
                        and ap[0][1] == 64):
                    a = pend[1]
                    aap = a["ins"][0]["ap"]
                    same = (a["ins"][0].get("memref") == inst["ins"][0].get("memref")
                            and aap[0][0] == ap[0][0] and aap[1] == ap[1]
                            and inst["ins"][0].get("offset", 0)
                            == a["ins"][0].get("offset", 0) + 64 * aap[0][0])
                    b_si = inst.get("sync_info") or {}
                    if same and not b_si.get("on_update"):
                        aap[0][1] = 128
                        a["tile_size"] = [128, 128]
                        a.setdefault("sync_info", {"on_update": [], "on_wait": []})
                        a["sync_info"].setdefault("on_wait", [])
                        a["sync_info"]["on_wait"].extend(b_si.get("on_wait") or [])
                        pend = None
                        continue
                out.append(inst)
                pend = None
            else:
                if op not in ("Matmult", "NoOp"):
                    pend = None
                out.append(inst)
        return out

    def patched(self, *a, **k):
        d = json.loads(orig(self, *a, **k))
        for f in d["functions"]:
            for bb in f["blocks"]:
                bb["instructions"] = _merge_ldw_halves(bb["instructions"])
                out = []
                last_ldw = None  # (key, still_valid)
                for inst in bb["instructions"]:
                    si = inst.get("sync_info")
                    ow = (si or {}).get("on_wait") or []
                    op = inst["opcode"]

                    def emit_carriers(waits):
                        for j, w in enumerate(waits):
                            out.append({
                                "name": f"{inst['name']}__w{j}",
                                "opcode": "NoOp",
                                "engine": inst["engine"],
                                "ins": [], "outs": [],
                                "debug": inst.get("debug", 0),
                                "sync_info": {"on_update": [], "on_wait": [w]},
                            })

                    # drop a Ldweights identical to the previous one when only
                    # Matmult/NoOp sit between (weights already resident);
                    # also fold the row-tiled [64,128]+[64,128] half-pair into
                    # the single [128,128] load emitted by _merge_ldw_halves
                    if op == "Ldweights" and inst["engine"] == "PE":
                        key = json.dumps(
                            [inst.get("ins"), inst.get("tile_position"),
                             inst.get("tile_size")], sort_keys=True)
                        if last_ldw == key and not (si or {}).get("on_update"):
                            emit_carriers(ow)
                            continue
                        last_ldw = key
                    elif inst["engine"] == "PE" and op not in ("Matmult", "NoOp"):
                        last_ldw = None

                    if len(ow) > 1:
                        emit_carriers(ow[:-1])
                        si["on_wait"] = [ow[-1]]
                    out.append(inst)
                bb["instructions"] = out
        return json.dumps(d).encode()

    bass.Bass.to_json_bytes = patched


def build_kernel():
    nc = bass.Bass()
    # host packs xT quarter-major: [tq*128+p, di*512+t] so each token
    # quarter is one contiguous [128, 4096] transfer with 4KB lines
    xT = nc.declare_dram_parameter("xT", [4 * 128, 8 * 512], BF16,
                                   isOutput=False)
    # wq|wk per-di rows (the first chains' critical bytes); wv separately,
    # host-packed [p, di*512+c] so it is one contiguous 8KB-line transfer
    wqk = nc.declare_dram_parameter("wqk", [DIM, 2 * FPC], BF16,
                                    isOutput=False)
    wv = nc.declare_dram_parameter("wv", [128, 8 * FPC], BF16,
                                   isOutput=False)
    wo = nc.declare_dram_parameter("wo", [FPC, DIM], BF16, isOutput=False)
    # tri = inclusive lower-triangular 0/1 mask; the diagonal S^T block is
    # exp'd unmasked (logits are bounded, no overflow) and the above-diag
    # entries are zeroed by a DVE multiply, keeping the mask off the PE
    tri = nc.declare_dram_parameter("tri", [128, 128], BF16, isOutput=False)
    outT = nc.declare_dram_parameter("outT", [DIM, SEQ], BF16, isOutput=True)

    with tile.TileContext(nc) as tc, ExitStack() as ctx:
        persist = ctx.enter_context(tc.tile_pool(name="persist", bufs=1))
        work = ctx.enter_context(tc.tile_pool(name="work", bufs=4))
        pt_pool = ctx.enter_context(tc.tile_pool(name="pt", bufs=1))
        ps_mm = ctx.enter_context(tc.tile_pool(name="ps_mm", bufs=2, space="PSUM"))
        ps_s = ctx.enter_context(tc.tile_pool(name="ps_s", bufs=2, space="PSUM"))
        ps_o = ctx.enter_context(tc.tile_pool(name="ps_o", bufs=2, space="PSUM"))

        # ---- load inputs. Fine-grained tiles so consumers start as soon
        # as their own bytes land: weights as per-di [wq|wk|wv] tiles
        # (contiguous 3KB lines) split across the two HW rings, xT as 4
        # host-preshuffled token-quarter tiles (contiguous 4KB lines),
        # Q/K as per-chunk tiles. The first QKV chains + sims are then
        # DMA-paced from ~13us instead of waiting for whole tensors. All
        # transfers keep coarse contiguous lines (fine-grained strided
        # patterns measurably downclock the chip via the power governor).
        w_sb = [persist.tile([128, 2 * FPC], BF16, tag=f"w{di}",
                             name=f"w{di}") for di in range(8)]
        wv_wide = persist.tile([128, 8 * FPC], BF16, tag="wv", name="wv")
        xq_sb = [persist.tile([128, 8 * 512], BF16, tag=f"xq{tq}",
                              name=f"xq{tq}") for tq in range(4)]
        wo_wide = persist.tile([128, 4 * DIM], BF16, tag="wo", name="wo")
        tri_sb = persist.tile([128, 128], BF16, tag="tri", name="tri")
        def wsl(name, di):       # [128,FPC] view of weight block di
            if name == "wv":
                return wv_wide[:, di * FPC:(di + 1) * FPC]
            return w_sb[di][:, 0:FPC] if name == "wq" else w_sb[di][:, FPC:]

        def xsl(di, tq):         # [128,512] token-quarter tq of x block di
            return xq_sb[tq][:, di * 512:(di + 1) * 512]

        def w_dma(eng, di):
            eng.dma_start(w_sb[di][:],
                          wqk.ap()[di * 128:(di + 1) * 128, :])

        def x_dma(eng, tq):
            eng.dma_start(xq_sb[tq][:],
                          xT.ap()[tq * 128:(tq + 1) * 128, :])

        # all weight blocks land by ~24us on the two HW rings; xq0 rides
        # the SWDGE front so the first chain has data ~20us; later token
        # quarters trail (their consumers are fills in later blocks)
        nc.scalar.dma_start(tri_sb[:], tri.ap())
        nc.gpsimd.dma_start(xq_sb[0][:], xT.ap()[0 * 128:1 * 128, :])
        w_dma(nc.sync, 0)
        w_dma(nc.scalar, 1)
        w_dma(nc.sync, 2)
        w_dma(nc.scalar, 3)
        w_dma(nc.sync, 4)
        w_dma(nc.scalar, 5)
        w_dma(nc.sync, 6)
        w_dma(nc.scalar, 7)
        nc.gpsimd.dma_start(wv_wide[:], wv.ap())
        x_dma(nc.scalar, 1)
        nc.gpsimd.dma_start(xq_sb[2][:], xT.ap()[2 * 128:3 * 128, :])
        x_dma(nc.sync, 3)
        nc.gpsimd.dma_start(
            wo_wide[:].rearrange("p (fi c) -> p fi c", fi=4),
            wo.ap().rearrange("(fi p) c -> p fi c", fi=4))
        wo_sb = [wo_wide[:, fi * DIM:(fi + 1) * DIM] for fi in range(4)]
        ones64 = persist.tile([1, DH], BF16, tag="ones64")
        nc.gpsimd.memset(ones64[:], 1.0)

        # ---- stage B: QKV projections -----------------------------------
        qk_sb = {"q": [], "k": []}
        for qn in ("q", "k"):
            for fi in range(4):
                qk_sb[qn].append(
                    [persist.tile([128, 512], BF16, tag=f"{qn}{fi}t{tck}",
                                  name=f"{qn}{fi}t{tck}") for tck in range(4)])
        v_sb = [persist.tile([128, HPC * VSTRIDE], BF16, tag=f"v{ti}",
                             name=f"v{ti}") for ti in range(NKT)]

        def emit_qk_part(qn, fi, tck):
            # one token-chunk of a pair's Q or K projection (fill unit)
            wn = "wq" if qn == "q" else "wk"
            ch = ps_mm.tile([128, 512], F32, tag="mm", name="qkp")
            for di in range(8):
                nc.tensor.matmul(
                    ch[:], wsl(wn, di)[:, fi * 128:(fi + 1) * 128],
                    xsl(di, tck),
                    start=(di == 0), stop=(di == 7))
            nc.vector.tensor_copy(qk_sb[qn][fi][tck][:], ch[:])

        def emit_v(ti):
            # V in [token, feature] layout (xT stationary, wv moving), strided
            # into VSTRIDE-blocks with a ones column per head
            t = v_sb[ti]
            p = ps_mm.tile([128, 512], F32, tag="mm", name="p_v")
            for di in range(8):
                nc.tensor.matmul(
                    p[:], xsl(di, ti // 4)[:, (ti % 4) * 128:(ti % 4 + 1) * 128],
                    wsl("wv", di),
                    start=(di == 0), stop=(di == 7))
            dst = t[:].rearrange("p (h c) -> p h c", h=HPC)[:, :, 0:DH]
            src = p[:].rearrange("p (h c) -> p h c", h=HPC)
            nc.vector.tensor_copy(dst, src)
            nc.gpsimd.memset(
                t[:].rearrange("p (h c) -> p h c", h=HPC)[:, :, DH:DH + 1], 1.0)

        ot_sb = [persist.tile([128, SEQ], BF16, tag=f"ot{fi}", name=f"ot{fi}")
                 for fi in range(4)]
        pts_map = {}

        def emit_sim(pr, ci, j0, j1):
            # S^T strips + exp into pt tiles for (head pair pr, q-chunk ci),
            # k-tiles j0..j1-1. Diagonal tiles (r >= 1) stream only their
            # valid q columns.
            q0 = ci * 512
            pts = pts_map.setdefault((pr, ci), {})
            for j in range(j0, j1):
                r = j - 4 * ci
                c0 = 128 * r if r > 0 else 0   # first valid q col in chunk
                ps = ps_s.tile([128, 1024], F32, tag="s", name="ps_st")
                for half in range(2):   # head A / head B, row-tiled
                    nc.tensor.matmul(
                        ps[:, half * 512 + c0:(half + 1) * 512],
                        qk_sb["k"][pr][j // 4][half * 64:(half + 1) * 64,
                                               (j % 4) * 128:(j % 4 + 1) * 128],
                        qk_sb["q"][pr][ci][half * 64:(half + 1) * 64,
                                           c0:512],
                        start=True, stop=True)
                pt = pt_pool.tile([128, 1024], BF16, tag=f"pt{j}", name="pt",
                                  bufs=2 if j < 14 else 1)
                pts[j] = pt
                if r < 0:
                    nc.scalar.activation(pt[:], ps[:], Exp, scale=SCALE)
                else:
                    # diagonal tile: exp the valid columns, then zero the
                    # above-diagonal entries of the in-block diagonal via a
                    # DVE mask-multiply (and the columns left of the valid
                    # range via memset — PV streams the full chunk on its
                    # closing matmul)
                    pt3 = pt[:].rearrange("p (b w) -> p b w", b=2)[:, :, c0:]
                    ps3 = ps[:].rearrange("p (b w) -> p b w", b=2)[:, :, c0:]
                    if r > 0:
                        nc.gpsimd.memset(
                            pt[:].rearrange("p (b w) -> p b w", b=2)[:, :, 0:c0],
                            0.0)
                    nc.scalar.activation(pt3, ps3, Exp, scale=SCALE)
                    for half in range(2):
                        ptd = pt[:, half * 512 + c0:half * 512 + c0 + 128]
                        nc.vector.tensor_mul(ptd, ptd, tri_sb[:])

        def emit_pv(pr, ci):
            # PV: V_aug stationary [128k, 65], P^T moving.
            # Output O^T_aug [65, 512q]: rows 0:64 = O^T, row 64 = sums.
            # Diagonal tiles r in {1,2} stream only valid columns; the last
            # tile streams full width (its masked cols are zero in pt) so
            # every PSUM element's accumulation closes with stop=True.
            # The UNNORMALIZED O^T is cast straight into ot_sb (freeing the
            # po bank as soon as the recip-input ln also reads it);
            # normalization happens in place two pairs later.
            q0 = ci * 512
            njs = 4 * ci + 4
            pts = pts_map.pop((pr, ci))
            pos = []
            # every diagonal tile (r>=1) streams only its valid columns:
            # the j=0 start=True clears has_written for the whole bank, so
            # elements the later partial-width tiles never touch keep their
            # earlier accumulated value, and the closing stop=True only
            # needs to cover its own columns
            for half in range(2):
                h = 2 * pr + half
                fi, row = h // 2, (h % 2) * 64
                po = ps_o.tile([DH + 1, 512], F32, tag="o", name="po")
                pos.append(po)
                for j in range(njs):
                    r = j - 4 * ci
                    c0 = 128 * r if r >= 1 else 0
                    nc.tensor.matmul(
                        po[:, c0:],
                        v_sb[j][:, h * VSTRIDE:h * VSTRIDE + DH + 1],
                        pts[j][:, half * 512 + c0:(half + 1) * 512],
                        start=(j == 0), stop=(j == njs - 1))
                nc.vector.tensor_copy(
                    ot_sb[fi][row:row + 64, q0:q0 + 512], po[0:DH, :])
            return [pr, ci, pos, None]

        def emit_recip(rec):
            # ln then exp(-x) of both sums rows (same ACT table set as the
            # softmax exps). Emitted inside the NEXT pair's exp stream so
            # the ACT never stalls waiting for the PV to finish.
            pr, ci, pos, _ = rec
            lrow = work.tile([1, 1024], F32, tag="lrow", name="lrow", bufs=1)
            rrow = work.tile([1, 1024], BF16, tag="rrow", name="rrow", bufs=2)
            for half in range(2):
                nc.scalar.activation(lrow[0:1, half * 512:(half + 1) * 512],
                                     pos[half][DH:DH + 1, :], Ln)
            nc.scalar.activation(rrow[:], lrow[:], Exp, scale=-1.0)
            rec[2] = None
            rec[3] = rrow

        def norm_finish(rec):
            # rank-1 matmuls broadcast each half's recip row across 64
            # partitions, then DVE scales O^T in place. Runs two pairs
            # after the PV, so the recip rows are always ready.
            pr, ci, _, rrow = rec
            q0 = ci * 512
            for half in range(2):
                h = 2 * pr + half
                fi, row = h // 2, (h % 2) * 64
                rb_ps = ps_mm.tile([DH, 512], F32, tag="mm", name="rb_ps")
                nc.tensor.matmul(
                    rb_ps[:], ones64[:],
                    rrow[0:1, half * 512:(half + 1) * 512],
                    start=True, stop=True)
                ot = ot_sb[fi][row:row + 64, q0:q0 + 512]
                nc.vector.tensor_mul(ot, ot, rb_ps[:])

        os2_box = [None]

        def emit_proj(ci, e0, e1):
            # projection for chunk ci's columns (all pairs' OT normalized).
            # Output cast into 2-ei-wide staging tiles; one batched DMA per
            # ei pair on the (otherwise idle) sync/vector queues so the
            # ACT queue never carries output-DMA issues.
            for ei in range(e0, e1):
                p = ps_mm.tile([128, 512], F32, tag="mm", name="p_proj")
                for fi in range(4):
                    nc.tensor.matmul(
                        p[:], wo_sb[fi][:, ei * 128:(ei + 1) * 128],
                        ot_sb[fi][:, ci * 512:(ci + 1) * 512],
                        start=(fi == 0), stop=(fi == 3))
                if ei % 2 == 0:
                    os2_box[0] = work.tile([128, 1024], BF16, tag="os2",
                                           name="os2", bufs=3)
                os2 = os2_box[0]
                nc.vector.tensor_copy(
                    os2[:, (ei % 2) * 512:(ei % 2 + 1) * 512], p[:])
                if ei % 2 == 1:
                    eng = nc.sync if ei % 4 == 1 else nc.gpsimd
                    eng.dma_start(
                        outT.ap()[(ei - 1) * 128:(ei + 1) * 128,
                                  ci * 512:(ci + 1) * 512]
                            .rearrange("(e p) c -> p e c", e=2),
                        os2[:].rearrange("p (e c) -> p e c", e=2))

        # Two-phase woven schedule balancing PE-heavy projection work
        # against the ACT-bound exp stream. Phase 1: per head-pair pr, its
        # chunks 0..2, with the NEXT pair's Q/K chains woven in (they only
        # use the mm psum tag, so they slot into exp-paced PE bubbles and
        # the next block's sims start without a projection stall); phase
        # 2: the four chunk-3 pairs, PE-filled with V group 3 and the
        # deferred output projections. Within a pair: off-diagonal sims,
        # previous pair's recip (ACT), V fills, pair n-2's norm_finish,
        # projection fill, Q/K weave, diagonal sims, PV + casts.
        # chunk-major block order: every block gets a uniform filler diet
        # (two deferred Q/K chunk-parts + one V tile), the DMA ramp gets
        # all four pairs' chunk-0 parts (they only need xq0 + wqk), and
        # the chunk projections spread mid-schedule instead of bunching
        # in the exp-bound endgame.
        order = [(pr, ci) for ci in range(4) for pr in range(4)]
        v_fill = {(0, 0): [0, 1, 2, 3], (1, 0): [4], (2, 0): [5],
                  (3, 0): [6], (0, 1): [7], (1, 1): [8], (2, 1): [9],
                  (3, 1): [10], (0, 2): [11], (1, 2): [12], (2, 2): [13],
                  (3, 2): [14], (0, 3): [15]}
        # the chunk-3 blocks are exp-bound (16 k-tiles of ACT work vs ~11
        # of PE); the mid-schedule is PE-dense on its own, so all three
        # ready projections land here as filler
        proj_fill = {(0, 3): 0, (1, 3): 1, (2, 3): 2}
        qk_parts = {
            (0, 0): [("q", 1, 0), ("k", 1, 0)],
            (1, 0): [("q", 2, 0), ("k", 2, 0)],
            (2, 0): [("q", 3, 0), ("k", 3, 0)],
            (3, 0): [("q", 0, 1), ("k", 0, 1)],
            (0, 1): [("q", 1, 1), ("k", 1, 1)],
            (1, 1): [("q", 2, 1), ("k", 2, 1)],
            (2, 1): [("q", 3, 1), ("k", 3, 1)],
            (3, 1): [("q", 0, 2), ("k", 0, 2)],
            (0, 2): [("q", 1, 2), ("k", 1, 2)],
            (1, 2): [("q", 2, 2), ("k", 2, 2)],
            (2, 2): [("q", 3, 2), ("k", 3, 2)],
            (3, 2): [("q", 0, 3), ("k", 0, 3)],
            (0, 3): [("q", 1, 3), ("k", 1, 3)],
            (1, 3): [("q", 2, 3), ("k", 2, 3)],
            (2, 3): [("q", 3, 3), ("k", 3, 3)],
        }
        # pair 0's chunk-0 Q/K only — the first sims start once these land;
        # the remaining pair-0 chunks ride the early blocks' fill slots so
        # the in-order PE queue never blocks on late token quarters
        emit_qk_part("q", 0, 0)
        emit_qk_part("k", 0, 0)
        pipe = []   # records awaiting recip (last) / norm_finish (first)
        for pr, ci in order:
            emit_sim(pr, ci, 0, 4 * ci)
            if pipe and pipe[-1][3] is None:
                emit_recip(pipe[-1])
            while len(pipe) >= 2:
                norm_finish(pipe.pop(0))
            if (pr, ci) in proj_fill:
                emit_proj(proj_fill[pr, ci], 0, 8)
            emit_sim(pr, ci, 4 * ci, 4 * ci + 4)
            for ti in v_fill.get((pr, ci), []):
                emit_v(ti)
            pipe.append(emit_pv(pr, ci))
            for qn, fi, tck in qk_parts.get((pr, ci), []):
                emit_qk_part(qn, fi, tck)
        emit_recip(pipe[-1])
        norm_finish(pipe.pop(0))
        norm_finish(pipe.pop(0))
        emit_proj(NQC - 1, 0, 8)
    return nc


_NC = None


def _get_nc():
    global _NC
    if _NC is None:
        _patch_walrus_wait_limit()
        _NC = build_kernel()
    return _NC


def _host_tri():
    # S^T orientation: rows = k tokens, cols = q tokens; valid iff q >= k
    return np.triu(np.ones((128, 128), dtype=np.float32)).astype(
        ml_dtypes.bfloat16)


def kernel(x, w_qkv, w_out, _trace=False, _trace_kwargs=None):
    x = np.asarray(x, dtype=np.float32)
    w_qkv = np.asarray(w_qkv, dtype=np.float32)
    w_out = np.asarray(w_out, dtype=np.float32)
    nc = _get_nc()

    tri = _host_tri()
    in_maps = []
    for c in range(NCORES):
        b, g = c // 2, c % 2
        cols = slice(g * FPC, (g + 1) * FPC)
        wq_ = w_qkv[:, 0 * DIM:1 * DIM][:, cols]
        wk_ = w_qkv[:, 1 * DIM:2 * DIM][:, cols]
        wv_ = w_qkv[:, 2 * DIM:3 * DIM][:, cols]
        # xT[di*128+p, tq*512+t] -> host[tq*128+p, di*512+t] so each token
        # quarter is a contiguous [128, 4096] block
        xt = np.ascontiguousarray(
            x[b].T.reshape(8, 128, 4, 512).transpose(2, 1, 0, 3)
                  .reshape(512, 4096))
        in_maps.append({
            "xT": xt.astype(ml_dtypes.bfloat16),
            "wqk": np.concatenate([wq_, wk_], axis=1)
                     .astype(ml_dtypes.bfloat16),
            "wv": np.ascontiguousarray(
                wv_.reshape(8, 128, 512).transpose(1, 0, 2)
                   .reshape(128, 4096)).astype(ml_dtypes.bfloat16),
            "wo": w_out[g * FPC:(g + 1) * FPC, :].astype(ml_dtypes.bfloat16),
            "tri": tri,
        })

    res = run_bass_kernel_spmd(
        nc, in_maps, core_ids=list(range(NCORES)),
        trace=_trace, **(_trace_kwargs or {}))
    out = np.empty((4, SEQ, DIM), dtype=np.float32)
    for b in range(4):
        out[b] = (res.results[2 * b]["outT"].astype(np.float32)
                  + res.results[2 * b + 1]["outT"].astype(np.float32)).T
    if _trace:
        kernel.last_results = res
    return out

